# revision 1
# baseline (speedup 1.0000x reference)
"""CapsNet dynamic-routing layer on 8 Trainium2 NeuronCores (Bass/Tile).

reference math (per batch element b):
  u_hat[b,i,o,j] = sum_d W[i,o,j,d] * u[b,i,d]        (never materialized)
  bl = 0; for r in 0..2:
    c = softmax_o(bl); s[b,o,j] = sum_i c*u_hat; v = squash(s)
    if r < 2: bl += sum_j u_hat*v
  return v  [B, 10, 16]

Distribution: pure data parallel, batch 512 -> 64 per core x 8 cores;
weights replicated.  Per-core: b=64, i=1152=9*128, o=10, j=16, d=8.

Key layout trick: o is mapped to PE column/row strips as g=o%4 (strip)
and sl=o//4 (slot), consistently across the s-matmuls (col-tiled),
squash (strip-local), the agreement matmuls (row-tiled), and the
output transposes - so no partition-moving shuffles are ever needed.
The softmax splits o as h=o//5 across partition halves (paired with
the agreement-pass PSUM packing) and o5=o%5 along free.
"""
import sys

sys.path.insert(0, "/opt/trn_rl_repo")

import numpy as np
import ml_dtypes
from contextlib import ExitStack

from concourse import bacc, mybir, hw_specs
from concourse.tile import TileContext
from concourse.bass_utils import run_bass_kernel_spmd

BF16 = mybir.dt.bfloat16
F32 = mybir.dt.float32
AX = mybir.AxisListType
ALU = mybir.AluOpType
ACTF = mybir.ActivationFunctionType
bfnp = ml_dtypes.bfloat16

B = 64
I = 1152
T = 9
O = 10
J = 16
D = 8
EPS = 1e-06
N_CORES = 8
KFLAT = D * I          # 9216 (d-major flat)
NCH = KFLAT // 512     # 18

_cache = {}

# Route every activation through the one table set that has exp+ln+copy,
# so the ACT engine never reloads tables mid-kernel.  Entry order (and
# hence act_func_set_id indices) is preserved.
_KEEP_SET = "natural_log_exp_and_others"


def _patched_tables(arch):
    full = {k: set(v) for k, v in hw_specs.get_activation_tables(arch).items()}
    keep = full[_KEEP_SET]
    return {k: (v if k == _KEEP_SET else v - keep) for k, v in full.items()}


import os
if os.environ.get('ACT_PATCH', '1') == '1':
    bacc.get_activation_tables = _patched_tables


def build_nc():
    nc = bacc.Bacc()
    ws_d = nc.dram_tensor("ws", [128, T, D, O, J], BF16, kind="ExternalInput")
    wb_d = nc.dram_tensor("wb", [128, 3 * KFLAT], BF16, kind="ExternalInput")
    ui_d = nc.dram_tensor("ui", [128, T, D, B], BF16, kind="ExternalInput")
    ur_d = nc.dram_tensor("ur", [128, KFLAT], BF16, kind="ExternalInput")
    cid2_d = nc.dram_tensor("cid2", [128, 64], F32, kind="ExternalInput")
    cid16s_d = nc.dram_tensor("cid16s", [128, 16], F32, kind="ExternalInput")
    cones128_d = nc.dram_tensor("cones128", [128, 1], F32, kind="ExternalInput")
    cones1_d = nc.dram_tensor("cones1", [1, 16], F32, kind="ExternalInput")
    vout_d = nc.dram_tensor("vout", [B, O, J], F32, kind="ExternalOutput")

    with TileContext(nc) as tc, ExitStack() as ctx:
        static = ctx.enter_context(tc.tile_pool(name="static", bufs=1))
        work = ctx.enter_context(tc.tile_pool(name="work", bufs=1))
        cupool = ctx.enter_context(tc.tile_pool(name="cup", bufs=2))
        psA = ctx.enter_context(tc.tile_pool(name="psA", bufs=1, space="PSUM"))
        psB = ctx.enter_context(tc.tile_pool(name="psB", bufs=3, space="PSUM"))
        psC = ctx.enter_context(tc.tile_pool(name="psC", bufs=2, space="PSUM"))
        psD = ctx.enter_context(tc.tile_pool(name="psD", bufs=2, space="PSUM"))

        ws = static.tile([128, T, D, O, J], BF16, name="ws")
        wb = static.tile([128, 3 * KFLAT], BF16, name="wb")
        ui = static.tile([128, T, D, B], BF16, name="ui")
        ur = static.tile([128, KFLAT], BF16, name="ur")
        cid2 = static.tile([128, 64], F32, name="cid2")
        cid16s = static.tile([128, 16], F32, name="cid16s")
        cones128 = static.tile([128, 1], F32, name="cones128")
        cones1 = static.tile([1, 16], F32, name="cones1")
        nc.sync.dma_start(out=ws, in_=ws_d[:, :, :, :, :])
        nc.sync.dma_start(out=wb, in_=wb_d[:, :])
        nc.sync.dma_start(out=ui, in_=ui_d[:, :, :, :])
        nc.sync.dma_start(out=ur, in_=ur_d[:, :])
        nc.sync.dma_start(out=cid2, in_=cid2_d[:, :])
        nc.sync.dma_start(out=cid16s, in_=cid16s_d[:, :])
        nc.sync.dma_start(out=cones128, in_=cones128_d[:, :])
        nc.sync.dma_start(out=cones1, in_=cones1_d[:, :])

        bl = work.tile([128, 5, I], F32, name="bl")
        c_t = work.tile([128, T, O, B], BF16, name="c_t")
        zh = work.tile([128, I], F32, name="zh")
        rz = work.tile([128, I], F32, name="rz")
        scratch = work.tile([128, KFLAT], BF16, name="scratch")
        ug = scratch
        e = scratch[:, 0 : 5 * I].rearrange("p (o i) -> p o i", o=5)
        s_sb = work.tile([128, 3, B], F32, name="s_sb")
        s2 = work.tile([128, 3, B], F32, name="s2")
        v_sb = work.tile([128, 3, B], F32, name="v_sb")
        v_st = work.tile([128, 3, B], BF16, name="v_st")
        sq_sb = work.tile([1, 4, 3, B], F32, name="sq_sb")
        t1p = work.tile([1, 768], F32, name="t1p")
        t2p = work.tile([1, 768], F32, name="t2p")
        den = work.tile([1, 768], F32, name="den")
        rec = work.tile([1, 768], F32, name="rec")
        v_t = work.tile([64, O, J], F32, name="v_t")
        eps1 = work.tile([1, 1], F32, name="eps1")
        nc.vector.memset(eps1, EPS)
        nc.vector.memset(s_sb.rearrange("p s b -> p (s b)"), 0.0)
        nc.vector.memset(sq_sb.rearrange("p g s b -> p (g s b)"), 0.0)

        PAIRS = [(0, 1), (2, 3), (4, 5), (6, 7), (8, 9)]

        def m1_pair(pair, rhs_of, scale):
            """col-tiled s matmuls for an o-pair -> s_sb strips."""
            ps = psA.tile([128, B], F32, name="m1ps", tag="m1ps")
            for t in range(T):
                for d in range(D):
                    for o in pair:
                        g = o % 4
                        nc.tensor.matmul(
                            ps[32 * g : 32 * g + 16, :],
                            ws[:, t, d, o, :],
                            rhs_of(o)[:, t, d, :],
                            start=(t == 0 and d == 0),
                            stop=(t == T - 1 and d == D - 1),
                            tile_position=(0, 32 * g),
                        )
            for o in pair:
                g, slot = o % 4, o // 4
                nc.scalar.mul(s_sb[32 * g : 32 * g + 16, slot, :],
                              ps[32 * g : 32 * g + 16, :], scale)

        def squash():
            """v_sb = squash(s_sb) with j on partitions (strip-local)."""
            sf = s_sb.rearrange("p s b -> p (s b)")
            s2f = s2.rearrange("p s b -> p (s b)")
            nc.vector.tensor_tensor(s2f, sf, sf, op=ALU.mult)
            for g in range(4):
                nsl = 3 if g < 2 else 2
                sqg = psD.tile([1, 3 * B], F32, name="sqg", tag="sqps")
                nc.tensor.matmul(
                    sqg[:, 0 : nsl * B],
                    cones128[32 * g : 32 * g + 16, :],
                    s2[32 * g : 32 * g + 16, 0:nsl, :],
                    start=True, stop=True,
                    tile_position=(32 * g, 0),
                )
                # scatter group's o-slices (o = g + 4*sl) into sq_sb
                nc.vector.tensor_copy(
                    sq_sb[:, g, 0:nsl, :],
                    sqg[:, 0 : nsl * B].rearrange("p (s b) -> p s b", s=nsl),
                )
            # o-major view of sq_sb: o = g + 4*sl  ->  dims (sl, g, b)
            sqv = sq_sb.transpose([0, 2, 1, 3])
            def _v(ap):
                return ap.rearrange("p (s g b) -> p s g b", s=3, g=4)
            nc.scalar.activation(_v(t1p), sqv, ACTF.Ln, bias=eps1)
            nc.scalar.activation(t2p, t1p, ACTF.Exp, scale=0.5)
            nc.vector.tensor_scalar_add(_v(den), sqv, 1.0)
            nc.vector.tensor_tensor(den, den, t2p, op=ALU.mult)
            nc.vector.reciprocal_approx_accurate(rec, den, t1p)
            nc.vector.tensor_tensor(_v(den), sqv, _v(rec), op=ALU.mult)
            mrep = psC.tile([128, 3 * B], F32, name="mrep", tag="miscps")
            nc.vector.memset(mrep, 0.0)
            for o in range(O):
                g, sl = o % 4, o // 4
                nc.tensor.matmul(
                    mrep[32 * g : 32 * g + 16, 64 * sl : 64 * (sl + 1)],
                    cones1,
                    den[:, 64 * o : 64 * (o + 1)],
                    start=True, stop=True,
                    tile_position=(0, 32 * g),
                )
            vf = v_sb.rearrange("p s b -> p (s b)")
            nc.vector.tensor_tensor(vf, sf, mrep, op=ALU.mult)

        def m2_b2(it):
            """bl (+)= sum_j u_hat * v   (g = W.T@v row-tiled; u*g; d-tree)."""
            nc.vector.tensor_copy(v_st.rearrange("p s b -> p (s b)"),
                                  v_sb.rearrange("p s b -> p (s b)"))
            for p in range(5):
                for n in range(NCH // 2):
                    for nn in (n, n + 9):
                        ps = psB.tile([128, 512], F32, name="m2ps", tag="m2ps")
                        for half, o in ((0, p), (1, p + 5)):
                            g, sl = o % 4, o // 4
                            nc.tensor.matmul(
                                ps[64 * half : 64 * half + 64, :],
                                v_st[32 * g : 32 * g + 16, sl, :],
                                wb[32 * g : 32 * g + 16,
                                   sl * KFLAT + 512 * nn : sl * KFLAT + 512 * (nn + 1)],
                                start=True, stop=True,
                                tile_position=(32 * g, 64 * half),
                            )
                        nc.scalar.copy(ug[:, 512 * nn : 512 * (nn + 1)], ps)
                    for nn in (n, n + 9):
                        sl = slice(512 * nn, 512 * (nn + 1))
                        nc.vector.tensor_tensor(ug[:, sl], ug[:, sl], ur[:, sl],
                                                op=ALU.mult)
                    sl = slice(512 * (n + 9), 512 * (n + 10))
                    nc.vector.tensor_tensor(
                        ug[:, sl], ug[:, 512 * n : 512 * (n + 1)],
                        ug[:, sl], op=ALU.add)
                h, q = KFLAT // 2, KFLAT // 4
                # l1 lives in [h:KFLAT); fold its halves into [h:h+q)
                nc.vector.tensor_tensor(ug[:, h : h + q], ug[:, h : h + q],
                                        ug[:, h + q : KFLAT], op=ALU.add)
                l3 = ug[:, h + q : h + q + 2 * I].bitcast(F32)
                nc.vector.tensor_tensor(l3, ug[:, h : h + I],
                                        ug[:, h + I : h + 2 * I], op=ALU.add)
                if it == 0:
                    nc.vector.tensor_copy(bl[:, p, :], l3)
                else:
                    nc.vector.tensor_tensor(bl[:, p, :], bl[:, p, :], l3,
                                            op=ALU.add)

        def softmax():
            """e := c = softmax_o(bl); c -> c_t (i-partitioned) via DMA-T."""
            nc.scalar.activation(e[:, :, :], bl[:, :, :], ACTF.Exp)
            nc.vector.tensor_tensor(zh, e[:, 0, :], e[:, 1, :], op=ALU.add)
            nc.vector.tensor_tensor(rz, e[:, 2, :], e[:, 3, :], op=ALU.add)
            nc.vector.tensor_tensor(zh, zh, e[:, 4, :], op=ALU.add)
            nc.vector.tensor_tensor(zh, zh, rz, op=ALU.add)
            for n in range(3):
                sl = slice(384 * n, 384 * (n + 1))
                zp = psC.tile([128, 384], F32, name="zswap", tag="miscps")
                nc.tensor.matmul(zp[0:64, :], cid2[64:128, :], zh[64:128, sl],
                                 start=True, stop=True, tile_position=(64, 0))
                nc.tensor.matmul(zp[64:128, :], cid2[0:64, :], zh[0:64, sl],
                                 start=True, stop=True, tile_position=(0, 64))
                nc.vector.tensor_tensor(zh[:, sl], zh[:, sl], zp, op=ALU.add)
            nc.vector.reciprocal_approx_fast(rz, zh)
            for o5 in range(5):
                nc.vector.tensor_tensor(e[:, o5, :], e[:, o5, :], rz,
                                        op=ALU.mult)
            for o in range(O):
                o5, hh = o % 5, o // 5
                for t in range(T):
                    nc.sync.dma_start_transpose(
                        out=c_t[:, t, o, :],
                        in_=e[64 * hh : 64 * hh + 64, o5,
                              128 * t : 128 * (t + 1)],
                    )

        # ========================= flow =========================
        import os as _os
        STAGE = int(_os.environ.get("FLOW_STAGE", "99"))
        for it in range(3):
            if it > 0 and STAGE < 4:
                break
            if it == 0:
                for pair in PAIRS:
                    m1_pair(pair, lambda o: ui, 0.1)
            else:
                for pair in PAIRS:
                    cus = {}
                    for o in pair:
                        cu = cupool.tile([128, T, D, B], BF16, name="cu",
                                         tag="cu")
                        nc.vector.tensor_tensor(
                            cu[:, :, :, :],
                            c_t[:, :, o, :].unsqueeze(2).broadcast_to(
                                [128, T, D, B]),
                            ui[:, :, :, :],
                            op=ALU.mult,
                        )
                        cus[o] = cu
                    m1_pair(pair, lambda o: cus[o], 1.0)
            if STAGE >= 1:
                squash()
            if it < 2 and STAGE >= 2:
                m2_b2(it)
                if STAGE >= 3:
                    softmax()

        if STAGE < 1:
            nc.vector.memset(v_sb.rearrange("p s b -> p (s b)"), 0.5)
        for o in range(O):
            g, sl = o % 4, o // 4
            tp = psC.tile([64, J], F32, name="vtp", tag="miscps")
            nc.tensor.transpose(tp, v_sb[32 * g : 32 * g + 16, sl, :],
                                cid16s[32 * g : 32 * g + 16, :],
                                tile_position=(32 * g, 0))
            nc.scalar.copy(v_t[:, o, :], tp)
        nc.sync.dma_start(out=vout_d[:, :, :], in_=v_t)

    nc.finalize()
    return nc


def _host_prep(u, weights):
    """Per-core input maps. u [512,1152,8] f32, weights [1152,10,16,8] f32."""
    W = np.asarray(weights, dtype=np.float32)
    u = np.asarray(u, dtype=np.float32)
    ws = np.ascontiguousarray(
        W.reshape(T, 128, O, J, D).transpose(1, 0, 4, 2, 3)
    ).astype(bfnp)  # [128, T, D, O, J]
    wt = W.transpose(1, 2, 3, 0)  # [o, j, d, i]
    wb = np.zeros((128, 3 * KFLAT), dtype=bfnp)
    for o in range(O):
        g, sl = o % 4, o // 4
        wb[32 * g : 32 * g + 16, sl * KFLAT : (sl + 1) * KFLAT] = (
            wt[o].reshape(J, KFLAT).astype(bfnp)
        )
    cid2 = np.concatenate([np.eye(64, dtype=np.float32)] * 2, axis=0)
    cid16s = np.zeros((128, 16), dtype=np.float32)
    for g in range(4):
        cid16s[32 * g : 32 * g + 16, :] = np.eye(16, dtype=np.float32)
    cones128 = np.ones((128, 1), dtype=np.float32)
    cones1 = np.ones((1, 16), dtype=np.float32)

    base = {
        "ws": ws, "wb": wb, "cid2": cid2, "cid16s": cid16s,
        "cones128": cones128, "cones1": cones1,
    }
    in_maps = []
    for c in range(N_CORES):
        uc = u[c * B : (c + 1) * B]  # [64, 1152, 8]
        ui = np.ascontiguousarray(
            uc.reshape(B, T, 128, D).transpose(2, 1, 3, 0)
        ).astype(bfnp)  # [128, T, D, B]
        urh = np.ascontiguousarray(uc.transpose(0, 2, 1)).reshape(B, KFLAT)
        ur = np.concatenate([urh, urh], axis=0).astype(bfnp)  # [128, KFLAT]
        in_maps.append({**base, "ui": ui, "ur": ur})
    return in_maps


def kernel(u, weights):
    if "nc" not in _cache:
        _cache["nc"] = build_nc()
    nc = _cache["nc"]
    in_maps = _host_prep(u, weights)
    res = run_bass_kernel_spmd(nc, in_maps, core_ids=list(range(N_CORES)))
    out = np.concatenate([res.results[c]["vout"] for c in range(N_CORES)], axis=0)
    return out.astype(np.float32)


if __name__ == "__main__":
    rng = np.random.default_rng(0)
    u = rng.standard_normal((512, 1152, 8), dtype=np.float32)
    w = (rng.standard_normal((1152, 10, 16, 8)) * 0.1).astype(np.float32)
    v = kernel(u, w)
    print("out", v.shape, v.dtype, np.abs(v).max())



# revision 2
# speedup vs baseline: 1.0008x; 1.0008x over previous
"""CapsNet dynamic-routing layer on 8 Trainium2 NeuronCores (Bass/Tile), v2.

reference math (per batch element b):
  u_hat[b,i,o,j] = sum_d W[i,o,j,d] * u[b,i,d]        (never materialized)
  bl = 0; for r in 0..2:
    c = softmax_o(bl); s[b,o,j] = sum_i c*u_hat; v = squash(s)
    if r < 2: bl += sum_j u_hat*v
  return v  [B, 10, 16]

Distribution: pure data parallel, batch 512 -> 64 per core x 8 cores;
weights replicated.  Per-core: b=64, i=1152=9*128, o=10, j=16, d=8.

v2 design vs v1:
  * m1 (s-matmuls) output b-partitioned [64, (o,j)] -> n=16/o per
    instruction instead of n=64: 4x fewer PE rows.
  * m2 (agreement) G^T = W^T v computed (d,i)-partitioned: out
    [128=(d,i)chunk, 64 b] per kc-chunk: 2x fewer PE rows than v1.
  * d-reduction done ON PE via chained identity matmuls accumulating
    in PSUM (start/stop over d) - removes the DVE add tree entirely.
  * logits bl never materialized: e2 = e1 * exp(delta2) folds the
    cross-iteration accumulation into the exp chain.
  * softmax stays i-partitioned end-to-end: no DMA transposes.
  * squash in b-partitioned layout: a handful of [64, 10]-sized ops.
  * PSUM->SBUF conversion work (f32 G -> bf16 for 2x-mode DVE mult)
    is routed per-o across Act / DVE-direct / Pool to balance engines.
"""
import sys

sys.path.insert(0, "/opt/trn_rl_repo")

import numpy as np
import ml_dtypes
from contextlib import ExitStack

from concourse import bacc, mybir, hw_specs
from concourse.tile import TileContext
from concourse.bass_utils import run_bass_kernel_spmd

BF16 = mybir.dt.bfloat16
F32 = mybir.dt.float32
AX = mybir.AxisListType
ALU = mybir.AluOpType
ACTF = mybir.ActivationFunctionType
bfnp = ml_dtypes.bfloat16

B = 64
I = 1152
T = 9          # i-chunks of 128 (also the softmax "c" index)
O = 10
J = 16
D = 8
KC = T * D     # 72 k-chunks of 128 over flat k = d*I + i
EPS = 1e-06
N_CORES = 8

_cache = {}

# Route every activation through the one table set that has exp+ln+copy,
# so the ACT engine never reloads tables mid-kernel.
_KEEP_SET = "natural_log_exp_and_others"


def _patched_tables(arch):
    full = {k: set(v) for k, v in hw_specs.get_activation_tables(arch).items()}
    keep = full[_KEEP_SET]
    return {k: (v if k == _KEEP_SET else v - keep) for k, v in full.items()}


import os
if os.environ.get('ACT_PATCH', '1') == '1':
    bacc.get_activation_tables = _patched_tables

# per-o conversion route for the m2 ug-mult (GPSIMD cannot read PSUM,
# so conversions are Act or DVE only):
#   'a' = Act copies PSUM f32 -> SBUF bf16, DVE multiplies at 2x
#   'A' = Act copies, Pool multiplies (SBUF-only, legal)
#   'b' = DVE multiplies straight from PSUM f32 (1x, no Act work)
#   'm' = per-chunk alternation: even chunks Act-copy, odd chunks
#         DVE-direct; DVE strided 2x mult for the copied half
ROUTES = os.environ.get("M2_ROUTES", "MMMMMMMMMM")
# per-o d-sum engine: 'T' PE identity-matmul chain (psum f32)
#                     'v' DVE in-place bf16 fold tree | 'g' Pool fold tree
DS_ENG = os.environ.get("DS_ENG", "TTgTvTTgTv")
# per-(it,o) cu engine: 'v' DVE | 'g' Pool  (20 chars: it1 o0-9, it2 o0-9)
CU_ENG = os.environ.get("CU_ENG", "gvgvgvgvgv" "gvgvgvgvgv")
# engine for per-o c = e*rz mult: 'v' DVE | 'g' Pool
C_ENG = os.environ.get("C_ENG", "v")
# per-o count of Act-copied chunk-groups in m/M routes (rest DVE-direct)
M_H = os.environ.get("M_H", "5454545454")
# engine for v_jb strip copies: 'a' Act | 'v' DVE
VJB_ENG = os.environ.get("VJB_ENG", "v")


def build_nc():
    nc = bacc.Bacc()
    ws_d = nc.dram_tensor("ws", [128, T, D, O, J], BF16, kind="ExternalInput")
    ui_d = nc.dram_tensor("ui", [128, T, D, B], BF16, kind="ExternalInput")
    ui2_d = nc.dram_tensor("ui2", [128, KC, B], BF16, kind="ExternalInput")
    wb2_d = nc.dram_tensor("wb2", [128, 3, KC, 128], BF16, kind="ExternalInput")
    id128_d = nc.dram_tensor("id128", [128, 128], BF16, kind="ExternalInput")
    id64_d = nc.dram_tensor("id64", [64, 64], BF16, kind="ExternalInput")
    vout_d = nc.dram_tensor("vout", [B, O, J], F32, kind="ExternalOutput")

    with TileContext(nc) as tc, ExitStack() as ctx:
        static = ctx.enter_context(tc.tile_pool(name="static", bufs=1))
        work = ctx.enter_context(tc.tile_pool(name="work", bufs=1))
        gsbp = ctx.enter_context(tc.tile_pool(name="gsbp", bufs=2))
        ugp = ctx.enter_context(tc.tile_pool(
            name="ugp", bufs=int(os.environ.get("DS_SHIFT", "3")) + 2))
        cup = ctx.enter_context(tc.tile_pool(name="cup", bufs=2))
        etp = ctx.enter_context(tc.tile_pool(name="etp", bufs=2))
        cop = ctx.enter_context(tc.tile_pool(name="cop", bufs=2))
        zp = ctx.enter_context(tc.tile_pool(name="zp", bufs=2))
        psS = ctx.enter_context(tc.tile_pool(name="psS", bufs=1, space="PSUM"))
        psVT = ctx.enter_context(tc.tile_pool(name="psVT", bufs=1, space="PSUM"))
        psG = ctx.enter_context(tc.tile_pool(
            name="psG", bufs=int(os.environ.get("PSG_BUFS", "4")), space="PSUM"))
        psDS = ctx.enter_context(tc.tile_pool(
            name="psDS", bufs=int(os.environ.get("PSDS_BUFS", "1")), space="PSUM"))

        # PE p-state: the clock ramps per continuous-busy stretch (reset on
        # idle; full speed only after 3us busy).  Dummy matmuls keep the PE
        # clock hot through DMA waits and phase boundaries.
        warm = static.tile([128, 128], BF16, name="warm")
        nc.vector.memset(warm, 0.0)

        def pe_keepalive(n):
            for _ in range(n):
                wps = psVT.tile([128, 96], F32, name="wps", tag="vt0")
                nc.tensor.matmul(wps, warm, warm[:, 0:96], start=True,
                                 stop=True, tile_position=(0, 0))

        pe_keepalive(int(os.environ.get("WARM0", "75")))

        ws = static.tile([128, T, D, O, J], BF16, name="ws")
        ui = static.tile([128, T, D, B], BF16, name="ui")
        ui2 = static.tile([128, KC, B], BF16, name="ui2")
        wb2 = static.tile([128, 3, KC, 128], BF16, name="wb2")
        id128 = static.tile([128, 128], BF16, name="id128")
        id64 = static.tile([64, 64], BF16, name="id64")
        eps1 = static.tile([64, 1], F32, name="eps1")
        nc.vector.memset(eps1, EPS)

        # DMA cost model (legacy CoreSim): each DMA holds the issuing
        # engine's queue for ~1.7us fixed + per-partition-free-bytes *
        # 0.39ns.  So: few big DMAs, spread across the SP / Act / Pool
        # queues, ordered by first use.
        # SP:   ws t0-4, id64, wb2 slot1, wb2 slot2
        # Pool: ws t5-8, wb2 slot0, id128
        # Act:  ui, ui2   (Act must be free for squash-0 at ~12us)
        if os.environ.get("DMA_PLAN", "A") == "A":
            nc.sync.dma_start(out=ws[:, 0:5], in_=ws_d[:, 0:5])
            nc.gpsimd.dma_start(out=ws[:, 5:9], in_=ws_d[:, 5:9])
            nc.scalar.dma_start(out=ui, in_=ui_d[:, :])
            nc.sync.dma_start(out=id64, in_=id64_d[:, :])
            nc.gpsimd.dma_start(out=wb2[:, 0], in_=wb2_d[:, 0])
            nc.gpsimd.dma_start(out=id128, in_=id128_d[:, :])
            nc.scalar.dma_start(out=ui2, in_=ui2_d[:, :])
            nc.sync.dma_start(out=wb2[:, 1], in_=wb2_d[:, 1])
            nc.sync.dma_start(out=wb2[:, 2], in_=wb2_d[:, 2])
        else:
            # plan D: ws in three pieces (SP x2, Pool x1) so the m1_A chain
            # streams behind the DMAs; slot0 follows on Pool just in time.
            nc.sync.dma_start(out=ws[:, 0:3], in_=ws_d[:, 0:3])
            nc.gpsimd.dma_start(out=ws[:, 3:6], in_=ws_d[:, 3:6])
            nc.scalar.dma_start(out=ui, in_=ui_d[:, :])
            nc.sync.dma_start(out=ws[:, 6:9], in_=ws_d[:, 6:9])
            nc.gpsimd.dma_start(out=wb2[:, 0], in_=wb2_d[:, 0])
            nc.sync.dma_start(out=id64, in_=id64_d[:, :])
            nc.gpsimd.dma_start(out=id128, in_=id128_d[:, :])
            nc.scalar.dma_start(out=ui2, in_=ui2_d[:, :])
            nc.sync.dma_start(out=wb2[:, 1], in_=wb2_d[:, 1])
            nc.sync.dma_start(out=wb2[:, 2], in_=wb2_d[:, 2])

        # persistent work tiles
        e = work.tile([128, O, T, B], BF16, name="e")
        rz_f = work.tile([128, T, B], F32, name="rz_f")
        rzb = work.tile([128, T, B], BF16, name="rzb")
        v_f = work.tile([64, O, J], F32, name="v_f")
        v_b = work.tile([64, O, J], BF16, name="v_b")
        v_jb = work.tile([128, 3, B], BF16, name="v_jb")
        s2 = work.tile([64, O, J], F32, name="s2")
        sq = work.tile([64, O], F32, name="sq")
        t1 = work.tile([64, O], F32, name="t1")
        den = work.tile([64, O], F32, name="den")
        rcp = work.tile([64, O], F32, name="rcp")
        ff = work.tile([64, O], F32, name="ff")

        s_ps = psS.tile([64, O, J], F32, name="s_ps")

        def m1_A_chain(h):
            """it0 half h: c uniform -> s_raw[b, o-half] = sum_{i,d} W u."""
            o5 = slice(5 * h, 5 * h + 5)
            for t in range(T):
                for d in range(D):
                    td = t * D + d
                    nc.tensor.matmul(
                        s_ps[:, o5, :].rearrange("p o j -> p (o j)"),
                        ui[:, t, d, :],
                        ws[:, t, d, o5, :].rearrange("p o j -> p (o j)"),
                        start=(td == 0), stop=(td == KC - 1),
                        tile_position=(0, 0), skip_group_check=True,
                    )

        def squash(it, h):
            """v[:, half] = squash(scale * s_ps[:, half]), tiny b-part ops."""
            scale = 0.1 if it == 0 else 1.0
            o5 = slice(5 * h, 5 * h + 5)
            nc.scalar.activation(s2[:, o5, :], s_ps[:, o5, :], ACTF.Square,
                                 scale=scale)
            nc.vector.tensor_reduce(sq[:, o5], s2[:, o5, :], axis=AX.X,
                                    op=ALU.add)
            nc.scalar.activation(t1[:, o5], sq[:, o5], ACTF.Ln, bias=eps1)
            nc.scalar.activation(den[:, o5], t1[:, o5], ACTF.Exp, scale=0.5)
            nc.vector.tensor_scalar_add(t1[:, o5], sq[:, o5], 1.0)
            nc.vector.tensor_tensor(den[:, o5], den[:, o5], t1[:, o5],
                                    op=ALU.mult)
            nc.vector.reciprocal(rcp[:, o5], den[:, o5])
            nc.vector.tensor_tensor(ff[:, o5], sq[:, o5], rcp[:, o5],
                                    op=ALU.mult)
            if it == 0:
                nc.vector.tensor_scalar_mul(ff[:, o5], ff[:, o5], scale)
            nc.vector.tensor_tensor(
                v_f[:, o5, :], s_ps[:, o5, :],
                ff[:, o5].unsqueeze(2).broadcast_to([64, 5, J]),
                op=ALU.mult)
            nc.vector.tensor_copy(
                v_b[:, o5, :].rearrange("p o j -> p (o j)"),
                v_f[:, o5, :].rearrange("p o j -> p (o j)"))

        # half-h (g, sl) slots are disjoint: h0 -> sl0 strips + (g0, sl1);
        # h1 -> (g1..3, sl1) + (g0..1, sl2).
        def transposes(h):
            vt = psVT.tile([128, 3, B], BF16, name="vt", tag="vt0",
                           bufs=1)
            for o in range(5 * h, 5 * h + 5):
                g, sl = o % 4, o // 4
                nc.tensor.matmul(
                    vt[32 * g : 32 * g + 16, sl, :],
                    v_b[:, o, :], id64,
                    is_transpose=True, tile_position=(0, 32 * g),
                )
            for o in range(5 * h, 5 * h + 5):
                g, sl = o % 4, o // 4
                if VJB_ENG == "a":
                    nc.scalar.copy(v_jb[32 * g : 32 * g + 16, sl, :],
                                   vt[32 * g : 32 * g + 16, sl, :])
                else:
                    nc.vector.tensor_copy(
                        v_jb[32 * g : 32 * g + 16, sl, :],
                        vt[32 * g : 32 * g + 16, sl, :])

        flat = lambda ap: ap.rearrange("p t b -> p (t b)")
        flat3 = lambda ap: ap.rearrange("p a b -> p (a b)")

        def emit_G(o, route):
            """G^T chunks for o; returns the ug tile being filled."""
            g, sl = o % 4, o // 4
            ug = ugp.tile([128, KC, B], BF16, name="ug", tag="ug")
            gsb = None
            if route != "b":
                gsb = gsbp.tile([128, KC, B], BF16, name="gsb", tag="gsb")

            def gmm(pg, kk, kc):
                nc.tensor.matmul(
                    pg[:, kk, :],
                    wb2[32 * g : 32 * g + 16, sl, kc, :],
                    v_jb[32 * g : 32 * g + 16, sl, :],
                    start=True, stop=True,
                    tile_position=(32 * g, 0),
                )

            if route == "6":
                # 16-kc psG tiles: (Act, Act, DVE, DVE, Act-half); Pool
                # multiplies the Act-copied parts.
                for ti in range(5):
                    k0 = 16 * ti
                    nk = 16 if ti < 4 else 8
                    pg = psG.tile([128, 16, B], F32, name="pg", tag="pg")
                    for kk in range(nk):
                        gmm(pg, kk, k0 + kk)
                    slk = slice(k0, k0 + nk)
                    if ti in (0, 1, 4):
                        nc.scalar.copy(flat3(gsb[:, slk, :]),
                                       flat3(pg[:, 0:nk, :]))
                    else:
                        nc.vector.tensor_tensor(
                            flat3(ug[:, slk, :]), flat3(pg[:, 0:nk, :]),
                            flat3(ui2[:, slk, :]), op=ALU.mult)
                nc.gpsimd.tensor_tensor(
                    flat3(ug[:, 0:32, :]), flat3(gsb[:, 0:32, :]),
                    flat3(ui2[:, 0:32, :]), op=ALU.mult)
                nc.gpsimd.tensor_tensor(
                    flat3(ug[:, 64:72, :]), flat3(gsb[:, 64:72, :]),
                    flat3(ui2[:, 64:72, :]), op=ALU.mult)
                return ug

            nA = int(M_H[o]) if route in ("m", "M") else 9
            for h in range(9):
                pg = psG.tile([128, 8, B], F32, name="pg", tag="pg")
                for kk in range(8):
                    gmm(pg, kk, 8 * h + kk)
                sl8 = slice(8 * h, 8 * h + 8)
                if route in ("a", "A") or (route in ("m", "M") and h < nA):
                    nc.scalar.copy(flat3(gsb[:, sl8, :]),
                                   flat3(pg[:, 0:8, :]))
                else:  # DVE straight from PSUM
                    nc.vector.tensor_tensor(
                        flat3(ug[:, sl8, :]), flat3(pg[:, 0:8, :]),
                        flat3(ui2[:, sl8, :]), op=ALU.mult)
            if route in ("a", "A"):
                meng = nc.gpsimd if route == "A" else nc.vector
                meng.tensor_tensor(flat3(ug), flat3(gsb), flat3(ui2),
                                   op=ALU.mult)
            elif route in ("m", "M"):
                hA = slice(0, 8 * nA)  # the Act-copied chunk-groups
                meng = nc.gpsimd if route == "M" else nc.vector
                meng.tensor_tensor(
                    flat3(ug[:, hA, :]), flat3(gsb[:, hA, :]),
                    flat3(ui2[:, hA, :]), op=ALU.mult)
            return ug

        def emit_ds(o, ug):
            """delta[o] = sum_d ug chunks.  Returns (psum_tile|None, ug)."""
            eng = DS_ENG[o]
            if eng == "T":  # PE identity-matmul chains into PSUM f32
                ds = psDS.tile([128, T, B], F32, name="ds", tag="ds")
                for d in range(D):
                    nc.tensor.matmul(
                        flat3(ds[:, 0:8, :]), id128,
                        flat3(ug[:, d * T : d * T + 8, :]),
                        start=(d == 0), stop=(d == D - 1),
                        tile_position=(0, 0), skip_group_check=True,
                    )
                for d in range(D):
                    nc.tensor.matmul(
                        ds[:, 8, :], id128, ug[:, d * T + 8, :],
                        start=(d == 0), stop=(d == D - 1),
                        tile_position=(0, 0), skip_group_check=True,
                    )
                return ds, ug
            ve = nc.vector if eng == "v" else nc.gpsimd
            # in-place bf16 fold tree: 72 -> 36 -> 18 -> 9 chunks
            for w in (36, 18, 9):
                ve.tensor_tensor(flat3(ug[:, 0:w, :]), flat3(ug[:, 0:w, :]),
                                 flat3(ug[:, w : 2 * w, :]), op=ALU.add)
            return None, ug

        def emit_exp(o, dsug, r):
            ds, ug = dsug
            src = flat3(ds) if ds is not None else flat3(ug[:, 0:T, :])
            if r == 0:
                nc.scalar.activation(flat(e[:, o]), src, ACTF.Exp)
            else:
                et = etp.tile([128, T, B], BF16, name="et", tag="et")
                nc.scalar.activation(flat(et), src, ACTF.Exp)
                nc.vector.tensor_tensor(flat(e[:, o]), flat(e[:, o]),
                                        flat(et), op=ALU.mult)

        def emit_zpair(q):
            """partial softmax sums on Pool, overlapped with m2."""
            zq = zp.tile([128, T, B], BF16, name="zq", tag=f"z{q}", bufs=1)
            nc.gpsimd.tensor_tensor(flat(zq), flat(e[:, 2 * q]),
                                    flat(e[:, 2 * q + 1]), op=ALU.add)
            _zpart.append(zq)
            if q in (1, 3):  # fold pairs into quads as soon as available
                zz = zp.tile([128, T, B], BF16, name="zz", tag=f"zz{q}",
                             bufs=1)
                nc.gpsimd.tensor_tensor(flat(zz), flat(_zpart[-2]),
                                        flat(_zpart[-1]), op=ALU.add)
                _zquad.append(zz)

        DS_SHIFT = int(os.environ.get("DS_SHIFT", "3"))

        def m2(r, it):
            """delta_o for all o -> e (pass r), software-pipelined.
            Caller has emitted squash(it,0)+transposes(0); squash/transposes
            of the second half are interleaved after G(1)."""
            ugs = {}
            dss = {}
            for step in range(O + DS_SHIFT + 1):
                if step < O:
                    ugs[step] = emit_G(step, ROUTES[step])
                if step == 1:
                    squash(it, 1)
                    transposes(1)
                if 0 <= step - DS_SHIFT < O:
                    dss[step - DS_SHIFT] = emit_ds(
                        step - DS_SHIFT, ugs.pop(step - DS_SHIFT))
                if 0 <= step - DS_SHIFT - 1 < O:
                    oo = step - DS_SHIFT - 1
                    emit_exp(oo, dss.pop(oo), r)
                    if oo % 2 == 1:
                        emit_zpair(oo // 2)
                if step == 3 and len(_zpart) >= 2:
                    pass

        def softmax_tail():
            """finish Z = sum_o e; rz = 1/Z (bf16)."""
            za = zp.tile([128, T, B], BF16, name="za", tag="za")
            nc.vector.tensor_tensor(flat(za), flat(_zquad[0]),
                                    flat(_zquad[1]), op=ALU.add)
            nc.vector.tensor_tensor(flat(za), flat(za), flat(_zpart[4]),
                                    op=ALU.add)
            with nc.allow_low_precision("softmax normalizer, 2e-2 tolerance"):
                nc.vector.reciprocal(flat(rzb), flat(za))
            _zpart.clear()
            _zquad.clear()

        def m1_B(it):
            """s[b, o, j] = sum_{i,d} (c_o u) W for all o.
            squash/transposes of half 0 are emitted after o=4's chain."""
            for o in range(O):
                co = cop.tile([128, T, B], BF16, name="co", tag="co")
                ceng = nc.gpsimd if C_ENG == "g" else nc.vector
                ceng.tensor_tensor(flat(co), flat(e[:, o]), flat(rzb),
                                   op=ALU.mult)
                cu = cup.tile([128, T, D, B], BF16, name="cu", tag="cu")
                cueng = nc.gpsimd if CU_ENG[(it - 1) * O + o] == "g" else nc.vector
                cueng.tensor_tensor(
                    cu[:, :, :, :],
                    co.unsqueeze(2).broadcast_to([128, T, D, B]),
                    ui[:, :, :, :], op=ALU.mult)
                for t in range(T):
                    for d in range(D):
                        td = t * D + d
                        nc.tensor.matmul(
                            s_ps[:, o, :], cu[:, t, d, :], ws[:, t, d, o, :],
                            start=(td == 0), stop=(td == KC - 1),
                            tile_position=(0, 0), skip_group_check=True,
                        )
                if o == 4:
                    squash(it, 0)
                    if it < 2:
                        transposes(0)
                    else:
                        nc.sync.dma_start(out=vout_d[:, 0:5, :],
                                          in_=v_f[:, 0:5, :])

        _zpart = []
        _zquad = []

        # ========================= flow =========================
        W1 = int(os.environ.get("WARM1", "0"))
        W2 = int(os.environ.get("WARM2", "0"))
        m1_A_chain(0)
        squash(0, 0)
        m1_A_chain(1)
        transposes(0)
        for r in range(2):
            m2(r, r)
            pe_keepalive(W1)
            softmax_tail()
            m1_B(r + 1)
            pe_keepalive(W2)
        squash(2, 1)
        nc.sync.dma_start(out=vout_d[:, 5:10, :], in_=v_f[:, 5:10, :])

    nc.finalize()
    return nc


def _host_prep(u, weights):
    """Per-core input maps. u [512,1152,8] f32, weights [1152,10,16,8] f32."""
    W = np.asarray(weights, dtype=np.float32)
    u = np.asarray(u, dtype=np.float32)
    # ws[p, t, d, o, j] = W[t*128+p, o, j, d]
    ws = np.ascontiguousarray(
        W.reshape(T, 128, O, J, D).transpose(1, 0, 4, 2, 3)
    ).astype(bfnp)
    # wb2[32g+jj, sl, kc, m] = W[c*128+m, o, jj, d], kc = d*T + c
    wt = W.reshape(T, 128, O, J, D)  # [c, m, o, j, d]
    wb2 = np.zeros((128, 3, KC, 128), dtype=bfnp)
    for o in range(O):
        g, sl = o % 4, o // 4
        blk = wt[:, :, o, :, :].transpose(2, 3, 0, 1)  # [j, d, c, m]
        wb2[32 * g : 32 * g + 16, sl] = blk.reshape(J, KC, 128).astype(bfnp)
    id128 = np.eye(128, dtype=np.float32).astype(bfnp)
    id64 = np.eye(64, dtype=np.float32).astype(bfnp)

    base = {"ws": ws, "wb2": wb2, "id128": id128, "id64": id64}
    in_maps = []
    for c in range(N_CORES):
        uc = u[c * B : (c + 1) * B]  # [64, 1152, 8]
        ur = uc.reshape(B, T, 128, D)
        ui = np.ascontiguousarray(ur.transpose(2, 1, 3, 0)).astype(bfnp)
        # ui2[p, kc, b] = u[b, c*128+p, d], kc = d*T + c
        ui2 = np.ascontiguousarray(
            ur.transpose(2, 3, 1, 0).reshape(128, D * T, B)
        ).astype(bfnp)
        in_maps.append({**base, "ui": ui, "ui2": ui2})
    return in_maps


def kernel(u, weights):
    if "nc" not in _cache:
        _cache["nc"] = build_nc()
    nc = _cache["nc"]
    in_maps = _host_prep(u, weights)
    res = run_bass_kernel_spmd(nc, in_maps, core_ids=list(range(N_CORES)))
    out = np.concatenate([res.results[c]["vout"] for c in range(N_CORES)], axis=0)
    return out.astype(np.float32)


if __name__ == "__main__":
    rng = np.random.default_rng(0)
    u = rng.standard_normal((512, 1152, 8), dtype=np.float32)
    w = (rng.standard_normal((1152, 10, 16, 8)) * 0.1).astype(np.float32)
    v = kernel(u, w)
    print("out", v.shape, v.dtype, np.abs(v).max())


# revision 3
# speedup vs baseline: 1.0030x; 1.0022x over previous
"""CapsNet dynamic-routing layer on 8 Trainium2 NeuronCores (Bass/Tile), v2.

reference math (per batch element b):
  u_hat[b,i,o,j] = sum_d W[i,o,j,d] * u[b,i,d]        (never materialized)
  bl = 0; for r in 0..2:
    c = softmax_o(bl); s[b,o,j] = sum_i c*u_hat; v = squash(s)
    if r < 2: bl += sum_j u_hat*v
  return v  [B, 10, 16]

Distribution: pure data parallel, batch 512 -> 64 per core x 8 cores;
weights replicated.  Per-core: b=64, i=1152=9*128, o=10, j=16, d=8.

v2 design vs v1:
  * m1 (s-matmuls) output b-partitioned [64, (o,j)] -> n=16/o per
    instruction instead of n=64: 4x fewer PE rows.
  * m2 (agreement) G^T = W^T v computed (d,i)-partitioned: out
    [128=(d,i)chunk, 64 b] per kc-chunk: 2x fewer PE rows than v1.
  * d-reduction done ON PE via chained identity matmuls accumulating
    in PSUM (start/stop over d) - removes the DVE add tree entirely.
  * logits bl never materialized: e2 = e1 * exp(delta2) folds the
    cross-iteration accumulation into the exp chain.
  * softmax stays i-partitioned end-to-end: no DMA transposes.
  * squash in b-partitioned layout: a handful of [64, 10]-sized ops.
  * PSUM->SBUF conversion work (f32 G -> bf16 for 2x-mode DVE mult)
    is routed per-o across Act / DVE-direct / Pool to balance engines.
"""
import sys

sys.path.insert(0, "/opt/trn_rl_repo")

import numpy as np
import ml_dtypes
from contextlib import ExitStack

from concourse import bacc, mybir, hw_specs
from concourse.tile import TileContext
from concourse.bass_utils import run_bass_kernel_spmd

BF16 = mybir.dt.bfloat16
F32 = mybir.dt.float32
AX = mybir.AxisListType
ALU = mybir.AluOpType
ACTF = mybir.ActivationFunctionType
bfnp = ml_dtypes.bfloat16

B = 64
I = 1152
T = 9          # i-chunks of 128 (also the softmax "c" index)
O = 10
J = 16
D = 8
KC = T * D     # 72 k-chunks of 128 over flat k = d*I + i
EPS = 1e-06
N_CORES = 8

_cache = {}

# Route every activation through the one table set that has exp+ln+copy,
# so the ACT engine never reloads tables mid-kernel.
_KEEP_SET = "natural_log_exp_and_others"


def _patched_tables(arch):
    full = {k: set(v) for k, v in hw_specs.get_activation_tables(arch).items()}
    keep = full[_KEEP_SET]
    return {k: (v if k == _KEEP_SET else v - keep) for k, v in full.items()}


import os
if os.environ.get('ACT_PATCH', '1') == '1':
    bacc.get_activation_tables = _patched_tables

# per-o conversion route for the m2 ug-mult (GPSIMD cannot read PSUM,
# so conversions are Act or DVE only):
#   'a' = Act copies PSUM f32 -> SBUF bf16, DVE multiplies at 2x
#   'A' = Act copies, Pool multiplies (SBUF-only, legal)
#   'b' = DVE multiplies straight from PSUM f32 (1x, no Act work)
#   'm' = per-chunk alternation: even chunks Act-copy, odd chunks
#         DVE-direct; DVE strided 2x mult for the copied half
ROUTES = os.environ.get("M2_ROUTES", "MMMMMMMMMM")
# per-o d-sum engine: 'T' PE identity-matmul chain (psum f32)
#                     'v' DVE in-place bf16 fold tree | 'g' Pool fold tree
DS_ENG = os.environ.get("DS_ENG", "TTgTvTTgTv")
# per-(it,o) cu engine: 'v' DVE | 'g' Pool  (20 chars: it1 o0-9, it2 o0-9)
CU_ENG = os.environ.get("CU_ENG", "vgvgvgvgvg" "gvgvgvgvgv")
# engine for per-o c = e*rz mult: 'v' DVE | 'g' Pool
C_ENG = os.environ.get("C_ENG", "v")
# per-o count of Act-copied chunk-groups in m/M routes (rest DVE-direct)
M_H = os.environ.get("M_H", "5454545454")
# engine for v_jb strip copies: 'a' Act | 'v' DVE
VJB_ENG = os.environ.get("VJB_ENG", "v")
# per-o engine for the pass-2 e = e*exp(delta) mult: 'v' DVE | 'g' Pool
E_ENG = os.environ.get("E_ENG", "gggggggggg")


def build_nc():
    nc = bacc.Bacc()
    ws_d = nc.dram_tensor("ws", [128, T, D, O, J], BF16, kind="ExternalInput")
    ui_d = nc.dram_tensor("ui", [128, T, D, B], BF16, kind="ExternalInput")
    ui2_d = nc.dram_tensor("ui2", [128, KC, B], BF16, kind="ExternalInput")
    wb2_d = nc.dram_tensor("wb2", [128, 3, KC, 128], BF16, kind="ExternalInput")
    id128_d = nc.dram_tensor("id128", [128, 128], BF16, kind="ExternalInput")
    id64_d = nc.dram_tensor("id64", [64, 64], BF16, kind="ExternalInput")
    vout_d = nc.dram_tensor("vout", [B, O, J], F32, kind="ExternalOutput")

    with TileContext(nc) as tc, ExitStack() as ctx:
        static = ctx.enter_context(tc.tile_pool(name="static", bufs=1))
        work = ctx.enter_context(tc.tile_pool(name="work", bufs=1))
        gsbp = ctx.enter_context(tc.tile_pool(name="gsbp", bufs=2))
        ugp = ctx.enter_context(tc.tile_pool(
            name="ugp", bufs=int(os.environ.get("DS_SHIFT", "3")) + 2))
        cup = ctx.enter_context(tc.tile_pool(name="cup", bufs=2))
        etp = ctx.enter_context(tc.tile_pool(name="etp", bufs=2))
        cop = ctx.enter_context(tc.tile_pool(name="cop", bufs=2))
        zp = ctx.enter_context(tc.tile_pool(name="zp", bufs=2))
        psS = ctx.enter_context(tc.tile_pool(name="psS", bufs=1, space="PSUM"))
        psVT = ctx.enter_context(tc.tile_pool(name="psVT", bufs=1, space="PSUM"))
        psG = ctx.enter_context(tc.tile_pool(
            name="psG", bufs=int(os.environ.get("PSG_BUFS", "4")), space="PSUM"))
        psDS = ctx.enter_context(tc.tile_pool(
            name="psDS", bufs=int(os.environ.get("PSDS_BUFS", "1")), space="PSUM"))

        # PE p-state: the clock ramps per continuous-busy stretch (reset on
        # idle; full speed only after 3us busy).  Dummy matmuls keep the PE
        # clock hot through DMA waits and phase boundaries.
        warm = static.tile([128, 128], BF16, name="warm")
        nc.vector.memset(warm, 0.0)

        def pe_keepalive(n):
            for _ in range(n):
                wps = psVT.tile([128, 96], F32, name="wps", tag="vt0")
                nc.tensor.matmul(wps, warm, warm[:, 0:96], start=True,
                                 stop=True, tile_position=(0, 0))

        pe_keepalive(int(os.environ.get("WARM0", "75")))

        ws = static.tile([128, T, D, O, J], BF16, name="ws")
        ui = static.tile([128, T, D, B], BF16, name="ui")
        ui2 = static.tile([128, KC, B], BF16, name="ui2")
        wb2 = static.tile([128, 3, KC, 128], BF16, name="wb2")
        id128 = static.tile([128, 128], BF16, name="id128")
        id64 = static.tile([64, 64], BF16, name="id64")
        eps1 = static.tile([64, 1], F32, name="eps1")
        nc.vector.memset(eps1, EPS)

        # DMA cost model (legacy CoreSim): each DMA holds the issuing
        # engine's queue for ~1.7us fixed + per-partition-free-bytes *
        # 0.39ns.  So: few big DMAs, spread across the SP / Act / Pool
        # queues, ordered by first use.
        # SP:   ws t0-4, id64, wb2 slot1, wb2 slot2
        # Pool: ws t5-8, wb2 slot0, id128
        # Act:  ui, ui2   (Act must be free for squash-0 at ~12us)
        if os.environ.get("DMA_PLAN", "A") == "A":
            nc.sync.dma_start(out=ws[:, 0:5], in_=ws_d[:, 0:5])
            nc.gpsimd.dma_start(out=ws[:, 5:9], in_=ws_d[:, 5:9])
            nc.scalar.dma_start(out=ui, in_=ui_d[:, :])
            nc.sync.dma_start(out=id64, in_=id64_d[:, :])
            nc.gpsimd.dma_start(out=wb2[:, 0], in_=wb2_d[:, 0])
            nc.gpsimd.dma_start(out=id128, in_=id128_d[:, :])
            nc.scalar.dma_start(out=ui2, in_=ui2_d[:, :])
            nc.sync.dma_start(out=wb2[:, 1], in_=wb2_d[:, 1])
            nc.sync.dma_start(out=wb2[:, 2], in_=wb2_d[:, 2])
        else:
            # plan D: ws in three pieces (SP x2, Pool x1) so the m1_A chain
            # streams behind the DMAs; slot0 follows on Pool just in time.
            nc.sync.dma_start(out=ws[:, 0:3], in_=ws_d[:, 0:3])
            nc.gpsimd.dma_start(out=ws[:, 3:6], in_=ws_d[:, 3:6])
            nc.scalar.dma_start(out=ui, in_=ui_d[:, :])
            nc.sync.dma_start(out=ws[:, 6:9], in_=ws_d[:, 6:9])
            nc.gpsimd.dma_start(out=wb2[:, 0], in_=wb2_d[:, 0])
            nc.sync.dma_start(out=id64, in_=id64_d[:, :])
            nc.gpsimd.dma_start(out=id128, in_=id128_d[:, :])
            nc.scalar.dma_start(out=ui2, in_=ui2_d[:, :])
            nc.sync.dma_start(out=wb2[:, 1], in_=wb2_d[:, 1])
            nc.sync.dma_start(out=wb2[:, 2], in_=wb2_d[:, 2])

        # persistent work tiles
        e = work.tile([128, O, T, B], BF16, name="e")
        rz_f = work.tile([128, T, B], F32, name="rz_f")
        rzb = work.tile([128, T, B], BF16, name="rzb")
        v_f = work.tile([64, O, J], F32, name="v_f")
        v_b = work.tile([64, O, J], BF16, name="v_b")
        v_jb = work.tile([128, 3, B], BF16, name="v_jb")
        s2 = work.tile([64, O, J], F32, name="s2")
        sq = work.tile([64, O], F32, name="sq")
        t1 = work.tile([64, O], F32, name="t1")
        den = work.tile([64, O], F32, name="den")
        rcp = work.tile([64, O], F32, name="rcp")
        ff = work.tile([64, O], F32, name="ff")

        s_ps = psS.tile([64, O, J], F32, name="s_ps")

        def m1_A_chain(h):
            """it0 half h: c uniform -> s_raw[b, o-half] = sum_{i,d} W u."""
            o5 = slice(5 * h, 5 * h + 5)
            for t in range(T):
                for d in range(D):
                    td = t * D + d
                    nc.tensor.matmul(
                        s_ps[:, o5, :].rearrange("p o j -> p (o j)"),
                        ui[:, t, d, :],
                        ws[:, t, d, o5, :].rearrange("p o j -> p (o j)"),
                        start=(td == 0), stop=(td == KC - 1),
                        tile_position=(0, 0), skip_group_check=True,
                    )

        def squash(it, h):
            """v[:, half] = squash(scale * s_ps[:, half]), tiny b-part ops."""
            scale = 0.1 if it == 0 else 1.0
            o5 = slice(5 * h, 5 * h + 5)
            nc.scalar.activation(s2[:, o5, :], s_ps[:, o5, :], ACTF.Square,
                                 scale=scale)
            nc.vector.tensor_reduce(sq[:, o5], s2[:, o5, :], axis=AX.X,
                                    op=ALU.add)
            nc.scalar.activation(t1[:, o5], sq[:, o5], ACTF.Ln, bias=eps1)
            nc.scalar.activation(den[:, o5], t1[:, o5], ACTF.Exp, scale=0.5)
            nc.vector.tensor_scalar_add(t1[:, o5], sq[:, o5], 1.0)
            nc.vector.tensor_tensor(den[:, o5], den[:, o5], t1[:, o5],
                                    op=ALU.mult)
            nc.vector.reciprocal(rcp[:, o5], den[:, o5])
            nc.vector.tensor_tensor(ff[:, o5], sq[:, o5], rcp[:, o5],
                                    op=ALU.mult)
            if it == 0:
                nc.vector.tensor_scalar_mul(ff[:, o5], ff[:, o5], scale)
            nc.vector.tensor_tensor(
                v_f[:, o5, :], s_ps[:, o5, :],
                ff[:, o5].unsqueeze(2).broadcast_to([64, 5, J]),
                op=ALU.mult)
            nc.vector.tensor_copy(
                v_b[:, o5, :].rearrange("p o j -> p (o j)"),
                v_f[:, o5, :].rearrange("p o j -> p (o j)"))

        # half-h (g, sl) slots are disjoint: h0 -> sl0 strips + (g0, sl1);
        # h1 -> (g1..3, sl1) + (g0..1, sl2).
        def transposes(h):
            vt = psVT.tile([128, 3, B], BF16, name="vt", tag="vt0",
                           bufs=1)
            for o in range(5 * h, 5 * h + 5):
                g, sl = o % 4, o // 4
                nc.tensor.matmul(
                    vt[32 * g : 32 * g + 16, sl, :],
                    v_b[:, o, :], id64,
                    is_transpose=True, tile_position=(0, 32 * g),
                )
            for o in range(5 * h, 5 * h + 5):
                g, sl = o % 4, o // 4
                if VJB_ENG == "a":
                    nc.scalar.copy(v_jb[32 * g : 32 * g + 16, sl, :],
                                   vt[32 * g : 32 * g + 16, sl, :])
                else:
                    nc.vector.tensor_copy(
                        v_jb[32 * g : 32 * g + 16, sl, :],
                        vt[32 * g : 32 * g + 16, sl, :])

        flat = lambda ap: ap.rearrange("p t b -> p (t b)")
        flat3 = lambda ap: ap.rearrange("p a b -> p (a b)")

        def emit_G(o, route):
            """G^T chunks for o; returns the ug tile being filled."""
            g, sl = o % 4, o // 4
            ug = ugp.tile([128, KC, B], BF16, name="ug", tag="ug")
            gsb = None
            if route != "b":
                gsb = gsbp.tile([128, KC, B], BF16, name="gsb", tag="gsb")

            def gmm(pg, kk, kc):
                nc.tensor.matmul(
                    pg[:, kk, :],
                    wb2[32 * g : 32 * g + 16, sl, kc, :],
                    v_jb[32 * g : 32 * g + 16, sl, :],
                    start=True, stop=True,
                    tile_position=(32 * g, 0),
                )

            if route == "6":
                # 16-kc psG tiles: (Act, Act, DVE, DVE, Act-half); Pool
                # multiplies the Act-copied parts.
                for ti in range(5):
                    k0 = 16 * ti
                    nk = 16 if ti < 4 else 8
                    pg = psG.tile([128, 16, B], F32, name="pg", tag="pg")
                    for kk in range(nk):
                        gmm(pg, kk, k0 + kk)
                    slk = slice(k0, k0 + nk)
                    if ti in (0, 1, 4):
                        nc.scalar.copy(flat3(gsb[:, slk, :]),
                                       flat3(pg[:, 0:nk, :]))
                    else:
                        nc.vector.tensor_tensor(
                            flat3(ug[:, slk, :]), flat3(pg[:, 0:nk, :]),
                            flat3(ui2[:, slk, :]), op=ALU.mult)
                nc.gpsimd.tensor_tensor(
                    flat3(ug[:, 0:32, :]), flat3(gsb[:, 0:32, :]),
                    flat3(ui2[:, 0:32, :]), op=ALU.mult)
                nc.gpsimd.tensor_tensor(
                    flat3(ug[:, 64:72, :]), flat3(gsb[:, 64:72, :]),
                    flat3(ui2[:, 64:72, :]), op=ALU.mult)
                return ug

            nA = int(M_H[o]) if route in ("m", "M") else 9
            for h in range(9):
                pg = psG.tile([128, 8, B], F32, name="pg", tag="pg")
                for kk in range(8):
                    gmm(pg, kk, 8 * h + kk)
                sl8 = slice(8 * h, 8 * h + 8)
                if route in ("a", "A") or (route in ("m", "M") and h < nA):
                    nc.scalar.copy(flat3(gsb[:, sl8, :]),
                                   flat3(pg[:, 0:8, :]))
                else:  # DVE straight from PSUM
                    nc.vector.tensor_tensor(
                        flat3(ug[:, sl8, :]), flat3(pg[:, 0:8, :]),
                        flat3(ui2[:, sl8, :]), op=ALU.mult)
            if route in ("a", "A"):
                meng = nc.gpsimd if route == "A" else nc.vector
                meng.tensor_tensor(flat3(ug), flat3(gsb), flat3(ui2),
                                   op=ALU.mult)
            elif route in ("m", "M"):
                hA = slice(0, 8 * nA)  # the Act-copied chunk-groups
                meng = nc.gpsimd if route == "M" else nc.vector
                meng.tensor_tensor(
                    flat3(ug[:, hA, :]), flat3(gsb[:, hA, :]),
                    flat3(ui2[:, hA, :]), op=ALU.mult)
            return ug

        def emit_ds(o, ug):
            """delta[o] = sum_d ug chunks.  Returns (psum_tile|None, ug)."""
            eng = DS_ENG[o]
            if eng == "T":  # PE identity-matmul chains into PSUM f32
                ds = psDS.tile([128, T, B], F32, name="ds", tag="ds")
                for d in range(D):
                    nc.tensor.matmul(
                        flat3(ds[:, 0:8, :]), id128,
                        flat3(ug[:, d * T : d * T + 8, :]),
                        start=(d == 0), stop=(d == D - 1),
                        tile_position=(0, 0), skip_group_check=True,
                    )
                for d in range(D):
                    nc.tensor.matmul(
                        ds[:, 8, :], id128, ug[:, d * T + 8, :],
                        start=(d == 0), stop=(d == D - 1),
                        tile_position=(0, 0), skip_group_check=True,
                    )
                return ds, ug
            ve = nc.vector if eng == "v" else nc.gpsimd
            # in-place bf16 fold tree: 72 -> 36 -> 18 -> 9 chunks
            for w in (36, 18, 9):
                ve.tensor_tensor(flat3(ug[:, 0:w, :]), flat3(ug[:, 0:w, :]),
                                 flat3(ug[:, w : 2 * w, :]), op=ALU.add)
            return None, ug

        def emit_exp(o, dsug, r):
            ds, ug = dsug
            src = flat3(ds) if ds is not None else flat3(ug[:, 0:T, :])
            if r == 0:
                nc.scalar.activation(flat(e[:, o]), src, ACTF.Exp)
            else:
                et = etp.tile([128, T, B], BF16, name="et", tag="et")
                nc.scalar.activation(flat(et), src, ACTF.Exp)
                eeng = nc.gpsimd if E_ENG[o] == "g" else nc.vector
                eeng.tensor_tensor(flat(e[:, o]), flat(e[:, o]),
                                   flat(et), op=ALU.mult)

        def emit_zpair(q):
            """partial softmax sums on Pool, overlapped with m2."""
            zq = zp.tile([128, T, B], BF16, name="zq", tag=f"z{q}", bufs=1)
            nc.gpsimd.tensor_tensor(flat(zq), flat(e[:, 2 * q]),
                                    flat(e[:, 2 * q + 1]), op=ALU.add)
            _zpart.append(zq)
            if q in (1, 3):  # fold pairs into quads as soon as available
                zz = zp.tile([128, T, B], BF16, name="zz", tag=f"zz{q}",
                             bufs=1)
                nc.gpsimd.tensor_tensor(flat(zz), flat(_zpart[-2]),
                                        flat(_zpart[-1]), op=ALU.add)
                _zquad.append(zz)

        DS_SHIFT = int(os.environ.get("DS_SHIFT", "3"))

        def m2(r, it):
            """delta_o for all o -> e (pass r), software-pipelined.
            Caller has emitted squash(it,0)+transposes(0); squash/transposes
            of the second half are interleaved after G(1)."""
            ugs = {}
            dss = {}
            for step in range(O + DS_SHIFT + 1):
                if step < O:
                    ugs[step] = emit_G(step, ROUTES[step])
                if step == 1:
                    squash(it, 1)
                    transposes(1)
                if 0 <= step - DS_SHIFT < O:
                    dss[step - DS_SHIFT] = emit_ds(
                        step - DS_SHIFT, ugs.pop(step - DS_SHIFT))
                if 0 <= step - DS_SHIFT - 1 < O:
                    oo = step - DS_SHIFT - 1
                    emit_exp(oo, dss.pop(oo), r)
                    if oo % 2 == 1:
                        emit_zpair(oo // 2)
                if step == 3 and len(_zpart) >= 2:
                    pass

        def softmax_tail():
            """finish Z = sum_o e; rz = 1/Z (bf16)."""
            za = zp.tile([128, T, B], BF16, name="za", tag="za")
            nc.vector.tensor_tensor(flat(za), flat(_zquad[0]),
                                    flat(_zquad[1]), op=ALU.add)
            nc.vector.tensor_tensor(flat(za), flat(za), flat(_zpart[4]),
                                    op=ALU.add)
            with nc.allow_low_precision("softmax normalizer, 2e-2 tolerance"):
                nc.vector.reciprocal(flat(rzb), flat(za))
            _zpart.clear()
            _zquad.clear()

        def m1_B(it):
            """s[b, o, j] = sum_{i,d} (c_o u) W for all o.
            squash/transposes of half 0 are emitted after o=4's chain."""
            for o in range(O):
                co = cop.tile([128, T, B], BF16, name="co", tag="co")
                ceng = nc.gpsimd if C_ENG == "g" else nc.vector
                ceng.tensor_tensor(flat(co), flat(e[:, o]), flat(rzb),
                                   op=ALU.mult)
                cu = cup.tile([128, T, D, B], BF16, name="cu", tag="cu")
                cueng = nc.gpsimd if CU_ENG[(it - 1) * O + o] == "g" else nc.vector
                cueng.tensor_tensor(
                    cu[:, :, :, :],
                    co.unsqueeze(2).broadcast_to([128, T, D, B]),
                    ui[:, :, :, :], op=ALU.mult)
                for t in range(T):
                    for d in range(D):
                        td = t * D + d
                        nc.tensor.matmul(
                            s_ps[:, o, :], cu[:, t, d, :], ws[:, t, d, o, :],
                            start=(td == 0), stop=(td == KC - 1),
                            tile_position=(0, 0), skip_group_check=True,
                        )
                if o == 4:
                    squash(it, 0)
                    if it < 2:
                        transposes(0)
                    else:
                        nc.sync.dma_start(out=vout_d[:, 0:5, :],
                                          in_=v_f[:, 0:5, :])

        _zpart = []
        _zquad = []

        # ========================= flow =========================
        W1 = int(os.environ.get("WARM1", "0"))
        W2 = int(os.environ.get("WARM2", "0"))
        m1_A_chain(0)
        squash(0, 0)
        m1_A_chain(1)
        transposes(0)
        for r in range(2):
            m2(r, r)
            pe_keepalive(W1)
            softmax_tail()
            m1_B(r + 1)
            pe_keepalive(W2)
        squash(2, 1)
        nc.sync.dma_start(out=vout_d[:, 5:10, :], in_=v_f[:, 5:10, :])

    nc.finalize()
    return nc


def _host_prep(u, weights):
    """Per-core input maps. u [512,1152,8] f32, weights [1152,10,16,8] f32."""
    W = np.asarray(weights, dtype=np.float32)
    u = np.asarray(u, dtype=np.float32)
    # ws[p, t, d, o, j] = W[t*128+p, o, j, d]
    ws = np.ascontiguousarray(
        W.reshape(T, 128, O, J, D).transpose(1, 0, 4, 2, 3)
    ).astype(bfnp)
    # wb2[32g+jj, sl, kc, m] = W[c*128+m, o, jj, d], kc = d*T + c
    wt = W.reshape(T, 128, O, J, D)  # [c, m, o, j, d]
    wb2 = np.zeros((128, 3, KC, 128), dtype=bfnp)
    for o in range(O):
        g, sl = o % 4, o // 4
        blk = wt[:, :, o, :, :].transpose(2, 3, 0, 1)  # [j, d, c, m]
        wb2[32 * g : 32 * g + 16, sl] = blk.reshape(J, KC, 128).astype(bfnp)
    id128 = np.eye(128, dtype=np.float32).astype(bfnp)
    id64 = np.eye(64, dtype=np.float32).astype(bfnp)

    base = {"ws": ws, "wb2": wb2, "id128": id128, "id64": id64}
    in_maps = []
    for c in range(N_CORES):
        uc = u[c * B : (c + 1) * B]  # [64, 1152, 8]
        ur = uc.reshape(B, T, 128, D)
        ui = np.ascontiguousarray(ur.transpose(2, 1, 3, 0)).astype(bfnp)
        # ui2[p, kc, b] = u[b, c*128+p, d], kc = d*T + c
        ui2 = np.ascontiguousarray(
            ur.transpose(2, 3, 1, 0).reshape(128, D * T, B)
        ).astype(bfnp)
        in_maps.append({**base, "ui": ui, "ui2": ui2})
    return in_maps


def kernel(u, weights):
    if "nc" not in _cache:
        _cache["nc"] = build_nc()
    nc = _cache["nc"]
    in_maps = _host_prep(u, weights)
    res = run_bass_kernel_spmd(nc, in_maps, core_ids=list(range(N_CORES)))
    out = np.concatenate([res.results[c]["vout"] for c in range(N_CORES)], axis=0)
    return out.astype(np.float32)


if __name__ == "__main__":
    rng = np.random.default_rng(0)
    u = rng.standard_normal((512, 1152, 8), dtype=np.float32)
    w = (rng.standard_normal((1152, 10, 16, 8)) * 0.1).astype(np.float32)
    v = kernel(u, w)
    print("out", v.shape, v.dtype, np.abs(v).max())


# revision 4
# speedup vs baseline: 1.0281x; 1.0250x over previous
"""CapsNet dynamic-routing layer on 8 Trainium2 NeuronCores (Bass/Tile), v2.

reference math (per batch element b):
  u_hat[b,i,o,j] = sum_d W[i,o,j,d] * u[b,i,d]        (never materialized)
  bl = 0; for r in 0..2:
    c = softmax_o(bl); s[b,o,j] = sum_i c*u_hat; v = squash(s)
    if r < 2: bl += sum_j u_hat*v
  return v  [B, 10, 16]

Distribution: pure data parallel, batch 512 -> 64 per core x 8 cores;
weights replicated.  Per-core: b=64, i=1152=9*128, o=10, j=16, d=8.

v2 design vs v1:
  * m1 (s-matmuls) output b-partitioned [64, (o,j)] -> n=16/o per
    instruction instead of n=64: 4x fewer PE rows.
  * m2 (agreement) G^T = W^T v computed (d,i)-partitioned: out
    [128=(d,i)chunk, 64 b] per kc-chunk: 2x fewer PE rows than v1.
  * d-reduction done ON PE via chained identity matmuls accumulating
    in PSUM (start/stop over d) - removes the DVE add tree entirely.
  * logits bl never materialized: e2 = e1 * exp(delta2) folds the
    cross-iteration accumulation into the exp chain.
  * softmax stays i-partitioned end-to-end: no DMA transposes.
  * squash in b-partitioned layout: a handful of [64, 10]-sized ops.
  * PSUM->SBUF conversion work (f32 G -> bf16 for 2x-mode DVE mult)
    is routed per-o across Act / DVE-direct / Pool to balance engines.
"""
import sys

sys.path.insert(0, "/opt/trn_rl_repo")

import numpy as np
import ml_dtypes
from contextlib import ExitStack

from concourse import bacc, mybir, hw_specs
from concourse.tile import TileContext
from concourse.bass_utils import run_bass_kernel_spmd

BF16 = mybir.dt.bfloat16
F32 = mybir.dt.float32
AX = mybir.AxisListType
ALU = mybir.AluOpType
ACTF = mybir.ActivationFunctionType
bfnp = ml_dtypes.bfloat16

B = 64
I = 1152
T = 9          # i-chunks of 128 (also the softmax "c" index)
O = 10
J = 16
D = 8
KC = T * D     # 72 k-chunks of 128 over flat k = d*I + i
EPS = 1e-06
N_CORES = 8

_cache = {}

# Route every activation through the one table set that has exp+ln+copy,
# so the ACT engine never reloads tables mid-kernel.
_KEEP_SET = "natural_log_exp_and_others"


def _patched_tables(arch):
    full = {k: set(v) for k, v in hw_specs.get_activation_tables(arch).items()}
    keep = full[_KEEP_SET]
    return {k: (v if k == _KEEP_SET else v - keep) for k, v in full.items()}


import os
if os.environ.get('ACT_PATCH', '1') == '1':
    bacc.get_activation_tables = _patched_tables

# per-o conversion route for the m2 ug-mult (GPSIMD cannot read PSUM,
# so conversions are Act or DVE only):
#   'a' = Act copies PSUM f32 -> SBUF bf16, DVE multiplies at 2x
#   'A' = Act copies, Pool multiplies (SBUF-only, legal)
#   'b' = DVE multiplies straight from PSUM f32 (1x, no Act work)
#   'm' = per-chunk alternation: even chunks Act-copy, odd chunks
#         DVE-direct; DVE strided 2x mult for the copied half
ROUTES = os.environ.get("M2_ROUTES", "MMMMMMMMMM")
# per-o d-sum engine: 'T' PE identity-matmul chain (psum f32)
#                     'v' DVE in-place bf16 fold tree | 'g' Pool fold tree
DS_ENG = os.environ.get("DS_ENG", "TTgTvTTgvT")
# per-(it,o) cu engine: 'v' DVE | 'g' Pool  (20 chars: it1 o0-9, it2 o0-9)
CU_ENG = os.environ.get("CU_ENG", "vgvvgvgvgv" "vgvvgvgvgv")
# engine for per-o c = e*rz mult: 'v' DVE | 'g' Pool
C_ENG = os.environ.get("C_ENG", "v")
# per-o count of Act-copied chunk-groups in m/M routes (rest DVE-direct)
M_H = os.environ.get("M_H", "5454545454")
# engine for v_jb strip copies: 'a' Act | 'v' DVE
VJB_ENG = os.environ.get("VJB_ENG", "v")
# per-o engine for the pass-2 e = e*exp(delta) mult: 'v' DVE | 'g' Pool
E_ENG = os.environ.get("E_ENG", "gggggggggg")


def build_nc():
    nc = bacc.Bacc()
    ws_d = nc.dram_tensor("ws", [128, T, D, O, J], BF16, kind="ExternalInput")
    ui_d = nc.dram_tensor("ui", [128, T, D, B], BF16, kind="ExternalInput")
    ui2_d = nc.dram_tensor("ui2", [128, KC, B], BF16, kind="ExternalInput")
    wb2_d = nc.dram_tensor("wb2", [128, 3, KC, 128], BF16, kind="ExternalInput")
    id128_d = nc.dram_tensor("id128", [128, 128], BF16, kind="ExternalInput")
    id64_d = nc.dram_tensor("id64", [64, 64], BF16, kind="ExternalInput")
    vout_d = nc.dram_tensor("vout", [B, O, J], F32, kind="ExternalOutput")

    with TileContext(nc) as tc, ExitStack() as ctx:
        static = ctx.enter_context(tc.tile_pool(name="static", bufs=1))
        work = ctx.enter_context(tc.tile_pool(name="work", bufs=1))
        gsbp = ctx.enter_context(tc.tile_pool(name="gsbp", bufs=2))
        ugp = ctx.enter_context(tc.tile_pool(
            name="ugp", bufs=int(os.environ.get("DS_SHIFT", "3")) + 2))
        cup = ctx.enter_context(tc.tile_pool(name="cup", bufs=2))
        etp = ctx.enter_context(tc.tile_pool(name="etp", bufs=2))
        cop = ctx.enter_context(tc.tile_pool(name="cop", bufs=2))
        zp = ctx.enter_context(tc.tile_pool(name="zp", bufs=2))
        psS = ctx.enter_context(tc.tile_pool(name="psS", bufs=1, space="PSUM"))
        psVT = ctx.enter_context(tc.tile_pool(name="psVT", bufs=1, space="PSUM"))
        psG = ctx.enter_context(tc.tile_pool(
            name="psG", bufs=int(os.environ.get("PSG_BUFS", "4")), space="PSUM"))
        psDS = ctx.enter_context(tc.tile_pool(
            name="psDS", bufs=int(os.environ.get("PSDS_BUFS", "1")), space="PSUM"))

        # PE p-state: the clock ramps per continuous-busy stretch (reset on
        # idle; full speed only after 3us busy).  Dummy matmuls keep the PE
        # clock hot through DMA waits and phase boundaries.
        warm = static.tile([128, 128], BF16, name="warm")
        nc.vector.memset(warm, 0.0)

        def pe_keepalive(n):
            for _ in range(n):
                wps = psVT.tile([128, 96], F32, name="wps", tag="vt0")
                nc.tensor.matmul(wps, warm, warm[:, 0:96], start=True,
                                 stop=True, tile_position=(0, 0))

        pe_keepalive(int(os.environ.get("WARM0", "75")))

        ws = static.tile([128, T, D, O, J], BF16, name="ws")
        ui = static.tile([128, T, D, B], BF16, name="ui")
        ui2 = static.tile([128, KC, B], BF16, name="ui2")
        wb2 = static.tile([128, 3, KC, 128], BF16, name="wb2")
        id128 = static.tile([128, 128], BF16, name="id128")
        id64 = static.tile([64, 64], BF16, name="id64")
        eps1 = static.tile([64, 1], F32, name="eps1")
        nc.vector.memset(eps1, EPS)

        # DMA cost model (legacy CoreSim): each DMA holds the issuing
        # engine's queue for ~1.7us fixed + per-partition-free-bytes *
        # 0.39ns.  So: few big DMAs, spread across the SP / Act / Pool
        # queues, ordered by first use.
        # SP:   ws t0-4, id64, wb2 slot1, wb2 slot2
        # Pool: ws t5-8, wb2 slot0, id128
        # Act:  ui, ui2   (Act must be free for squash-0 at ~12us)
        if os.environ.get("DMA_PLAN", "A") == "A":
            nc.sync.dma_start(out=ws[:, 0:5], in_=ws_d[:, 0:5])
            nc.gpsimd.dma_start(out=ws[:, 5:9], in_=ws_d[:, 5:9])
            nc.scalar.dma_start(out=ui, in_=ui_d[:, :])
            nc.sync.dma_start(out=id64, in_=id64_d[:, :])
            nc.gpsimd.dma_start(out=wb2[:, 0], in_=wb2_d[:, 0])
            nc.gpsimd.dma_start(out=id128, in_=id128_d[:, :])
            nc.scalar.dma_start(out=ui2, in_=ui2_d[:, :])
            nc.sync.dma_start(out=wb2[:, 1], in_=wb2_d[:, 1])
            nc.sync.dma_start(out=wb2[:, 2], in_=wb2_d[:, 2])
        else:
            # plan E: per-t ws chunks alternating SP/Pool so the m1_A chain
            # streams at DMA pitch; slot0 split across both queues after.
            for t in range(0, 9, 2):
                nc.sync.dma_start(out=ws[:, t], in_=ws_d[:, t])
            for t in range(1, 9, 2):
                nc.gpsimd.dma_start(out=ws[:, t], in_=ws_d[:, t])
            nc.scalar.dma_start(out=ui, in_=ui_d[:, :])
            nc.scalar.dma_start(out=id64, in_=id64_d[:, :])
            nc.sync.dma_start(out=wb2[:, 0, 0:36], in_=wb2_d[:, 0, 0:36])
            nc.gpsimd.dma_start(out=wb2[:, 0, 36:72], in_=wb2_d[:, 0, 36:72])
            nc.scalar.dma_start(out=ui2, in_=ui2_d[:, :])
            nc.sync.dma_start(out=id128, in_=id128_d[:, :])
            nc.sync.dma_start(out=wb2[:, 1], in_=wb2_d[:, 1])
            nc.gpsimd.dma_start(out=wb2[:, 2], in_=wb2_d[:, 2])

        # persistent work tiles
        e = work.tile([128, O, T, B], BF16, name="e")
        rz_f = work.tile([128, T, B], F32, name="rz_f")
        rzb = work.tile([128, T, B], BF16, name="rzb")
        v_f = work.tile([64, O, J], F32, name="v_f")
        v_b = work.tile([64, O, J], BF16, name="v_b")
        v_jb = work.tile([128, 3, B], BF16, name="v_jb")
        s2 = work.tile([64, O, J], F32, name="s2")
        sq = work.tile([64, O], F32, name="sq")
        t1 = work.tile([64, O], F32, name="t1")
        den = work.tile([64, O], F32, name="den")
        rcp = work.tile([64, O], F32, name="rcp")
        ff = work.tile([64, O], F32, name="ff")

        s_ps = psS.tile([64, O, J], F32, name="s_ps")

        def m1_A_chain(h):
            """it0 half h: c uniform -> s_raw[b, o-half] = sum_{i,d} W u."""
            o5 = slice(5 * h, 5 * h + 5)
            for t in range(T):
                for d in range(D):
                    td = t * D + d
                    nc.tensor.matmul(
                        s_ps[:, o5, :].rearrange("p o j -> p (o j)"),
                        ui[:, t, d, :],
                        ws[:, t, d, o5, :].rearrange("p o j -> p (o j)"),
                        start=(td == 0), stop=(td == KC - 1),
                        tile_position=(0, 0), skip_group_check=True,
                    )

        def squash(it, h):
            """v[:, half] = squash(scale * s_ps[:, half]), tiny b-part ops."""
            scale = 0.1 if it == 0 else 1.0
            o5 = slice(5 * h, 5 * h + 5)
            nc.scalar.activation(s2[:, o5, :], s_ps[:, o5, :], ACTF.Square,
                                 scale=scale)
            nc.vector.tensor_reduce(sq[:, o5], s2[:, o5, :], axis=AX.X,
                                    op=ALU.add)
            nc.scalar.activation(t1[:, o5], sq[:, o5], ACTF.Ln, bias=eps1)
            nc.scalar.activation(den[:, o5], t1[:, o5], ACTF.Exp, scale=0.5)
            nc.vector.tensor_scalar_add(t1[:, o5], sq[:, o5], 1.0)
            nc.vector.tensor_tensor(den[:, o5], den[:, o5], t1[:, o5],
                                    op=ALU.mult)
            nc.vector.reciprocal(rcp[:, o5], den[:, o5])
            nc.vector.tensor_tensor(ff[:, o5], sq[:, o5], rcp[:, o5],
                                    op=ALU.mult)
            if it == 0:
                nc.vector.tensor_scalar_mul(ff[:, o5], ff[:, o5], scale)
            nc.vector.tensor_tensor(
                v_f[:, o5, :], s_ps[:, o5, :],
                ff[:, o5].unsqueeze(2).broadcast_to([64, 5, J]),
                op=ALU.mult)
            nc.vector.tensor_copy(
                v_b[:, o5, :].rearrange("p o j -> p (o j)"),
                v_f[:, o5, :].rearrange("p o j -> p (o j)"))

        # half-h (g, sl) slots are disjoint: h0 -> sl0 strips + (g0, sl1);
        # h1 -> (g1..3, sl1) + (g0..1, sl2).
        def transposes(h):
            vt = psVT.tile([128, 3, B], BF16, name="vt", tag="vt0",
                           bufs=1)
            for o in range(5 * h, 5 * h + 5):
                g, sl = o % 4, o // 4
                nc.tensor.matmul(
                    vt[32 * g : 32 * g + 16, sl, :],
                    v_b[:, o, :], id64,
                    is_transpose=True, tile_position=(0, 32 * g),
                )
            for o in range(5 * h, 5 * h + 5):
                g, sl = o % 4, o // 4
                if VJB_ENG == "a":
                    nc.scalar.copy(v_jb[32 * g : 32 * g + 16, sl, :],
                                   vt[32 * g : 32 * g + 16, sl, :])
                else:
                    nc.vector.tensor_copy(
                        v_jb[32 * g : 32 * g + 16, sl, :],
                        vt[32 * g : 32 * g + 16, sl, :])

        flat = lambda ap: ap.rearrange("p t b -> p (t b)")
        flat3 = lambda ap: ap.rearrange("p a b -> p (a b)")

        def emit_G(o, route):
            """G^T chunks for o; returns the ug tile being filled."""
            g, sl = o % 4, o // 4
            ug = ugp.tile([128, KC, B], BF16, name="ug", tag="ug")
            gsb = None
            if route != "b":
                gsb = gsbp.tile([128, KC, B], BF16, name="gsb", tag="gsb")

            def gmm(pg, kk, kc):
                nc.tensor.matmul(
                    pg[:, kk, :],
                    wb2[32 * g : 32 * g + 16, sl, kc, :],
                    v_jb[32 * g : 32 * g + 16, sl, :],
                    start=True, stop=True,
                    tile_position=(32 * g, 0),
                )

            if route == "6":
                # 16-kc psG tiles: (Act, Act, DVE, DVE, Act-half); Pool
                # multiplies the Act-copied parts.
                for ti in range(5):
                    k0 = 16 * ti
                    nk = 16 if ti < 4 else 8
                    pg = psG.tile([128, 16, B], F32, name="pg", tag="pg")
                    for kk in range(nk):
                        gmm(pg, kk, k0 + kk)
                    slk = slice(k0, k0 + nk)
                    if ti in (0, 1, 4):
                        nc.scalar.copy(flat3(gsb[:, slk, :]),
                                       flat3(pg[:, 0:nk, :]))
                    else:
                        nc.vector.tensor_tensor(
                            flat3(ug[:, slk, :]), flat3(pg[:, 0:nk, :]),
                            flat3(ui2[:, slk, :]), op=ALU.mult)
                nc.gpsimd.tensor_tensor(
                    flat3(ug[:, 0:32, :]), flat3(gsb[:, 0:32, :]),
                    flat3(ui2[:, 0:32, :]), op=ALU.mult)
                nc.gpsimd.tensor_tensor(
                    flat3(ug[:, 64:72, :]), flat3(gsb[:, 64:72, :]),
                    flat3(ui2[:, 64:72, :]), op=ALU.mult)
                return ug

            nA = int(M_H[o]) if route in ("m", "M") else 9
            for h in range(9):
                pg = psG.tile([128, 8, B], F32, name="pg", tag="pg")
                for kk in range(8):
                    gmm(pg, kk, 8 * h + kk)
                sl8 = slice(8 * h, 8 * h + 8)
                if route in ("a", "A") or (route in ("m", "M") and h < nA):
                    nc.scalar.copy(flat3(gsb[:, sl8, :]),
                                   flat3(pg[:, 0:8, :]))
                else:  # DVE straight from PSUM
                    nc.vector.tensor_tensor(
                        flat3(ug[:, sl8, :]), flat3(pg[:, 0:8, :]),
                        flat3(ui2[:, sl8, :]), op=ALU.mult)
            if route in ("a", "A"):
                meng = nc.gpsimd if route == "A" else nc.vector
                meng.tensor_tensor(flat3(ug), flat3(gsb), flat3(ui2),
                                   op=ALU.mult)
            elif route in ("m", "M"):
                hA = slice(0, 8 * nA)  # the Act-copied chunk-groups
                meng = nc.gpsimd if route == "M" else nc.vector
                meng.tensor_tensor(
                    flat3(ug[:, hA, :]), flat3(gsb[:, hA, :]),
                    flat3(ui2[:, hA, :]), op=ALU.mult)
            return ug

        def emit_ds(o, ug):
            """delta[o] = sum_d ug chunks.  Returns (psum_tile|None, ug)."""
            eng = DS_ENG[o]
            if eng == "T":  # PE identity-matmul chains into PSUM f32
                ds = psDS.tile([128, T, B], F32, name="ds", tag="ds")
                for d in range(D):
                    nc.tensor.matmul(
                        flat3(ds[:, 0:8, :]), id128,
                        flat3(ug[:, d * T : d * T + 8, :]),
                        start=(d == 0), stop=(d == D - 1),
                        tile_position=(0, 0), skip_group_check=True,
                    )
                for d in range(D):
                    nc.tensor.matmul(
                        ds[:, 8, :], id128, ug[:, d * T + 8, :],
                        start=(d == 0), stop=(d == D - 1),
                        tile_position=(0, 0), skip_group_check=True,
                    )
                return ds, ug
            ve = nc.vector if eng == "v" else nc.gpsimd
            # in-place bf16 fold tree: 72 -> 36 -> 18 -> 9 chunks
            for w in (36, 18, 9):
                ve.tensor_tensor(flat3(ug[:, 0:w, :]), flat3(ug[:, 0:w, :]),
                                 flat3(ug[:, w : 2 * w, :]), op=ALU.add)
            return None, ug

        def emit_exp(o, dsug, r):
            ds, ug = dsug
            src = flat3(ds) if ds is not None else flat3(ug[:, 0:T, :])
            if r == 0:
                nc.scalar.activation(flat(e[:, o]), src, ACTF.Exp)
            else:
                et = etp.tile([128, T, B], BF16, name="et", tag="et")
                nc.scalar.activation(flat(et), src, ACTF.Exp)
                eeng = nc.gpsimd if E_ENG[o] == "g" else nc.vector
                eeng.tensor_tensor(flat(e[:, o]), flat(e[:, o]),
                                   flat(et), op=ALU.mult)

        def emit_zpair(q):
            """partial softmax sums on Pool, overlapped with m2."""
            zq = zp.tile([128, T, B], BF16, name="zq", tag=f"z{q}", bufs=1)
            nc.gpsimd.tensor_tensor(flat(zq), flat(e[:, 2 * q]),
                                    flat(e[:, 2 * q + 1]), op=ALU.add)
            _zpart.append(zq)
            if q in (1, 3):  # fold pairs into quads as soon as available
                zz = zp.tile([128, T, B], BF16, name="zz", tag=f"zz{q}",
                             bufs=1)
                nc.gpsimd.tensor_tensor(flat(zz), flat(_zpart[-2]),
                                        flat(_zpart[-1]), op=ALU.add)
                _zquad.append(zz)

        DS_SHIFT = int(os.environ.get("DS_SHIFT", "3"))

        def m2(r, it):
            """delta_o for all o -> e (pass r), software-pipelined.
            Caller has emitted squash(it,0)+transposes(0); squash/transposes
            of the second half are interleaved after G(1)."""
            ugs = {}
            dss = {}
            for step in range(O + DS_SHIFT + 1):
                if step < O:
                    ugs[step] = emit_G(step, ROUTES[step])
                if step == 1:
                    squash(it, 1)
                    transposes(1)
                if 0 <= step - DS_SHIFT < O:
                    dss[step - DS_SHIFT] = emit_ds(
                        step - DS_SHIFT, ugs.pop(step - DS_SHIFT))
                if 0 <= step - DS_SHIFT - 1 < O:
                    oo = step - DS_SHIFT - 1
                    emit_exp(oo, dss.pop(oo), r)
                    if oo % 2 == 1:
                        emit_zpair(oo // 2)
                if step == 3 and len(_zpart) >= 2:
                    pass

        def softmax_tail():
            """finish Z = sum_o e; rz = 1/Z (bf16)."""
            za = zp.tile([128, T, B], BF16, name="za", tag="za")
            nc.vector.tensor_tensor(flat(za), flat(_zquad[0]),
                                    flat(_zquad[1]), op=ALU.add)
            nc.vector.tensor_tensor(flat(za), flat(za), flat(_zpart[4]),
                                    op=ALU.add)
            with nc.allow_low_precision("softmax normalizer, 2e-2 tolerance"):
                nc.vector.reciprocal(flat(rzb), flat(za))
            _zpart.clear()
            _zquad.clear()

        def m1_B(it):
            """s[b, o, j] = sum_{i,d} (c_o u) W for all o.
            squash/transposes of half 0 are emitted after o=4's chain."""
            for o in range(O):
                co = cop.tile([128, T, B], BF16, name="co", tag="co")
                ceng = nc.gpsimd if C_ENG == "g" else nc.vector
                ceng.tensor_tensor(flat(co), flat(e[:, o]), flat(rzb),
                                   op=ALU.mult)
                cu = cup.tile([128, T, D, B], BF16, name="cu", tag="cu")
                cueng = nc.gpsimd if CU_ENG[(it - 1) * O + o] == "g" else nc.vector
                cueng.tensor_tensor(
                    cu[:, :, :, :],
                    co.unsqueeze(2).broadcast_to([128, T, D, B]),
                    ui[:, :, :, :], op=ALU.mult)
                for t in range(T):
                    for d in range(D):
                        td = t * D + d
                        nc.tensor.matmul(
                            s_ps[:, o, :], cu[:, t, d, :], ws[:, t, d, o, :],
                            start=(td == 0), stop=(td == KC - 1),
                            tile_position=(0, 0), skip_group_check=True,
                        )
                if o == 4:
                    squash(it, 0)
                    if it < 2:
                        transposes(0)
                    else:
                        nc.sync.dma_start(out=vout_d[:, 0:5, :],
                                          in_=v_f[:, 0:5, :])

        _zpart = []
        _zquad = []

        # ========================= flow =========================
        W1 = int(os.environ.get("WARM1", "0"))
        W2 = int(os.environ.get("WARM2", "0"))
        m1_A_chain(0)
        squash(0, 0)
        m1_A_chain(1)
        transposes(0)
        for r in range(2):
            m2(r, r)
            pe_keepalive(W1)
            softmax_tail()
            m1_B(r + 1)
            pe_keepalive(W2)
        squash(2, 1)
        nc.sync.dma_start(out=vout_d[:, 5:10, :], in_=v_f[:, 5:10, :])

    nc.finalize()
    return nc


def _host_prep(u, weights):
    """Per-core input maps. u [512,1152,8] f32, weights [1152,10,16,8] f32."""
    W = np.asarray(weights, dtype=np.float32)
    u = np.asarray(u, dtype=np.float32)
    # ws[p, t, d, o, j] = W[t*128+p, o, j, d]
    ws = np.ascontiguousarray(
        W.reshape(T, 128, O, J, D).transpose(1, 0, 4, 2, 3)
    ).astype(bfnp)
    # wb2[32g+jj, sl, kc, m] = W[c*128+m, o, jj, d], kc = d*T + c
    wt = W.reshape(T, 128, O, J, D)  # [c, m, o, j, d]
    wb2 = np.zeros((128, 3, KC, 128), dtype=bfnp)
    for o in range(O):
        g, sl = o % 4, o // 4
        blk = wt[:, :, o, :, :].transpose(2, 3, 0, 1)  # [j, d, c, m]
        wb2[32 * g : 32 * g + 16, sl] = blk.reshape(J, KC, 128).astype(bfnp)
    id128 = np.eye(128, dtype=np.float32).astype(bfnp)
    id64 = np.eye(64, dtype=np.float32).astype(bfnp)

    base = {"ws": ws, "wb2": wb2, "id128": id128, "id64": id64}
    in_maps = []
    for c in range(N_CORES):
        uc = u[c * B : (c + 1) * B]  # [64, 1152, 8]
        ur = uc.reshape(B, T, 128, D)
        ui = np.ascontiguousarray(ur.transpose(2, 1, 3, 0)).astype(bfnp)
        # ui2[p, kc, b] = u[b, c*128+p, d], kc = d*T + c
        ui2 = np.ascontiguousarray(
            ur.transpose(2, 3, 1, 0).reshape(128, D * T, B)
        ).astype(bfnp)
        in_maps.append({**base, "ui": ui, "ui2": ui2})
    return in_maps


def kernel(u, weights):
    if "nc" not in _cache:
        _cache["nc"] = build_nc()
    nc = _cache["nc"]
    in_maps = _host_prep(u, weights)
    res = run_bass_kernel_spmd(nc, in_maps, core_ids=list(range(N_CORES)))
    out = np.concatenate([res.results[c]["vout"] for c in range(N_CORES)], axis=0)
    return out.astype(np.float32)


if __name__ == "__main__":
    rng = np.random.default_rng(0)
    u = rng.standard_normal((512, 1152, 8), dtype=np.float32)
    w = (rng.standard_normal((1152, 10, 16, 8)) * 0.1).astype(np.float32)
    v = kernel(u, w)
    print("out", v.shape, v.dtype, np.abs(v).max())


# revision 5
# speedup vs baseline: 1.0495x; 1.0208x over previous
"""CapsNet dynamic-routing layer on 8 Trainium2 NeuronCores (Bass/Tile), v2.

reference math (per batch element b):
  u_hat[b,i,o,j] = sum_d W[i,o,j,d] * u[b,i,d]        (never materialized)
  bl = 0; for r in 0..2:
    c = softmax_o(bl); s[b,o,j] = sum_i c*u_hat; v = squash(s)
    if r < 2: bl += sum_j u_hat*v
  return v  [B, 10, 16]

Distribution: pure data parallel, batch 512 -> 64 per core x 8 cores;
weights replicated.  Per-core: b=64, i=1152=9*128, o=10, j=16, d=8.

v2 design vs v1:
  * m1 (s-matmuls) output b-partitioned [64, (o,j)] -> n=16/o per
    instruction instead of n=64: 4x fewer PE rows.
  * m2 (agreement) G^T = W^T v computed (d,i)-partitioned: out
    [128=(d,i)chunk, 64 b] per kc-chunk: 2x fewer PE rows than v1.
  * d-reduction done ON PE via chained identity matmuls accumulating
    in PSUM (start/stop over d) - removes the DVE add tree entirely.
  * logits bl never materialized: e2 = e1 * exp(delta2) folds the
    cross-iteration accumulation into the exp chain.
  * softmax stays i-partitioned end-to-end: no DMA transposes.
  * squash in b-partitioned layout: a handful of [64, 10]-sized ops.
  * PSUM->SBUF conversion work (f32 G -> bf16 for 2x-mode DVE mult)
    is routed per-o across Act / DVE-direct / Pool to balance engines.
"""
import sys

sys.path.insert(0, "/opt/trn_rl_repo")

import numpy as np
import ml_dtypes
from contextlib import ExitStack

from concourse import bacc, mybir, hw_specs
from concourse.tile import TileContext
from concourse.bass_utils import run_bass_kernel_spmd

BF16 = mybir.dt.bfloat16
F32 = mybir.dt.float32
AX = mybir.AxisListType
ALU = mybir.AluOpType
ACTF = mybir.ActivationFunctionType
bfnp = ml_dtypes.bfloat16

B = 64
I = 1152
T = 9          # i-chunks of 128 (also the softmax "c" index)
O = 10
J = 16
D = 8
KC = T * D     # 72 k-chunks of 128 over flat k = d*I + i
EPS = 1e-06
N_CORES = 8

_cache = {}

# Route every activation through the one table set that has exp+ln+copy,
# so the ACT engine never reloads tables mid-kernel.
_KEEP_SET = "natural_log_exp_and_others"


def _patched_tables(arch):
    full = {k: set(v) for k, v in hw_specs.get_activation_tables(arch).items()}
    keep = full[_KEEP_SET]
    return {k: (v if k == _KEEP_SET else v - keep) for k, v in full.items()}


import os
if os.environ.get('ACT_PATCH', '1') == '1':
    bacc.get_activation_tables = _patched_tables

# per-o conversion route for the m2 ug-mult (GPSIMD cannot read PSUM,
# so conversions are Act or DVE only):
#   'a' = Act copies PSUM f32 -> SBUF bf16, DVE multiplies at 2x
#   'A' = Act copies, Pool multiplies (SBUF-only, legal)
#   'b' = DVE multiplies straight from PSUM f32 (1x, no Act work)
#   'm' = per-chunk alternation: even chunks Act-copy, odd chunks
#         DVE-direct; DVE strided 2x mult for the copied half
ROUTES = os.environ.get("M2_ROUTES", "MMMMMMMMMM")
# per-o d-sum engine: 'T' PE identity-matmul chain (psum f32)
#                     'v' DVE in-place bf16 fold tree | 'g' Pool fold tree
DS_ENG = os.environ.get("DS_ENG", "TgTTvTTgvT")
# per-(it,o) cu engine: 'v' DVE | 'g' Pool  (20 chars: it1 o0-9, it2 o0-9)
CU_ENG = os.environ.get("CU_ENG", "vgvvgvgvgv" "vgvvgvgvgv")
# engine for per-o c = e*rz mult: 'v' DVE | 'g' Pool
C_ENG = os.environ.get("C_ENG", "gv" * 10)
if len(C_ENG) == 1:
    C_ENG = C_ENG * 20
# per-o count of Act-copied chunk-groups in m/M routes (rest DVE-direct)
M_H = os.environ.get("M_H", "5455545445")
# engine for v_jb strip copies: 'a' Act | 'v' DVE
VJB_ENG = os.environ.get("VJB_ENG", "v")
# per-o engine for the pass-2 e = e*exp(delta) mult: 'v' DVE | 'g' Pool
E_ENG = os.environ.get("E_ENG", "vgvgvgvgvg")


def build_nc():
    nc = bacc.Bacc()
    ws_d = nc.dram_tensor("ws", [128, T, D, O, J], BF16, kind="ExternalInput")
    ui_d = nc.dram_tensor("ui", [128, T, D, B], BF16, kind="ExternalInput")
    ui2_d = nc.dram_tensor("ui2", [128, KC, B], BF16, kind="ExternalInput")
    wb2_d = nc.dram_tensor("wb2", [128, 3, KC, 128], BF16, kind="ExternalInput")
    id128_d = nc.dram_tensor("id128", [128, 128], BF16, kind="ExternalInput")
    id64_d = nc.dram_tensor("id64", [64, 64], BF16, kind="ExternalInput")
    vout_d = nc.dram_tensor("vout", [B, O, J], F32, kind="ExternalOutput")

    with TileContext(nc) as tc, ExitStack() as ctx:
        static = ctx.enter_context(tc.tile_pool(name="static", bufs=1))
        work = ctx.enter_context(tc.tile_pool(name="work", bufs=1))
        gsbp = ctx.enter_context(tc.tile_pool(name="gsbp", bufs=2))
        ugp = ctx.enter_context(tc.tile_pool(
            name="ugp", bufs=int(os.environ.get("DS_SHIFT", "3")) + 2))
        cup = ctx.enter_context(tc.tile_pool(name="cup", bufs=2))
        etp = ctx.enter_context(tc.tile_pool(name="etp", bufs=2))
        cop = ctx.enter_context(tc.tile_pool(name="cop", bufs=2))
        zp = ctx.enter_context(tc.tile_pool(name="zp", bufs=2))
        psS = ctx.enter_context(tc.tile_pool(name="psS", bufs=1, space="PSUM"))
        psVT = ctx.enter_context(tc.tile_pool(name="psVT", bufs=1, space="PSUM"))
        psG = ctx.enter_context(tc.tile_pool(
            name="psG", bufs=int(os.environ.get("PSG_BUFS", "4")), space="PSUM"))
        psDS = ctx.enter_context(tc.tile_pool(
            name="psDS", bufs=int(os.environ.get("PSDS_BUFS", "1")), space="PSUM"))

        # PE p-state: the clock ramps per continuous-busy stretch (reset on
        # idle; full speed only after 3us busy).  Dummy matmuls keep the PE
        # clock hot through DMA waits and phase boundaries.
        warm = static.tile([128, 128], BF16, name="warm")
        nc.vector.memset(warm, 0.0)

        def pe_keepalive(n):
            for _ in range(n):
                wps = psVT.tile([128, 96], F32, name="wps", tag="vt0")
                nc.tensor.matmul(wps, warm, warm[:, 0:96], start=True,
                                 stop=True, tile_position=(0, 0))

        pe_keepalive(int(os.environ.get("WARM0", "75")))

        ws = static.tile([128, T, D, O, J], BF16, name="ws")
        ui = static.tile([128, T, D, B], BF16, name="ui")
        ui2 = static.tile([128, KC, B], BF16, name="ui2")
        wb2 = static.tile([128, 3, KC, 128], BF16, name="wb2")
        id128 = static.tile([128, 128], BF16, name="id128")
        id64 = static.tile([64, 64], BF16, name="id64")
        eps1 = static.tile([64, 1], F32, name="eps1")
        nc.vector.memset(eps1, EPS)

        # DMA cost model (legacy CoreSim): each DMA holds the issuing
        # engine's queue for ~1.7us fixed + per-partition-free-bytes *
        # 0.39ns.  So: few big DMAs, spread across the SP / Act / Pool
        # queues, ordered by first use.
        # SP:   ws t0-4, id64, wb2 slot1, wb2 slot2
        # Pool: ws t5-8, wb2 slot0, id128
        # Act:  ui, ui2   (Act must be free for squash-0 at ~12us)
        if os.environ.get("DMA_PLAN", "A") == "A":
            nc.sync.dma_start(out=ws[:, 0:5], in_=ws_d[:, 0:5])
            nc.gpsimd.dma_start(out=ws[:, 5:9], in_=ws_d[:, 5:9])
            nc.scalar.dma_start(out=ui, in_=ui_d[:, :])
            nc.sync.dma_start(out=id64, in_=id64_d[:, :])
            nc.gpsimd.dma_start(out=wb2[:, 0], in_=wb2_d[:, 0])
            nc.gpsimd.dma_start(out=id128, in_=id128_d[:, :])
            nc.scalar.dma_start(out=ui2, in_=ui2_d[:, :])
            nc.sync.dma_start(out=wb2[:, 1], in_=wb2_d[:, 1])
            nc.sync.dma_start(out=wb2[:, 2], in_=wb2_d[:, 2])
        else:
            # plan E: per-t ws chunks alternating SP/Pool so the m1_A chain
            # streams at DMA pitch; slot0 split across both queues after.
            for t in range(0, 9, 2):
                nc.sync.dma_start(out=ws[:, t], in_=ws_d[:, t])
            for t in range(1, 9, 2):
                nc.gpsimd.dma_start(out=ws[:, t], in_=ws_d[:, t])
            nc.scalar.dma_start(out=ui, in_=ui_d[:, :])
            nc.scalar.dma_start(out=id64, in_=id64_d[:, :])
            nc.sync.dma_start(out=wb2[:, 0, 0:36], in_=wb2_d[:, 0, 0:36])
            nc.gpsimd.dma_start(out=wb2[:, 0, 36:72], in_=wb2_d[:, 0, 36:72])
            nc.scalar.dma_start(out=ui2, in_=ui2_d[:, :])
            nc.sync.dma_start(out=id128, in_=id128_d[:, :])
            nc.sync.dma_start(out=wb2[:, 1], in_=wb2_d[:, 1])
            nc.gpsimd.dma_start(out=wb2[:, 2], in_=wb2_d[:, 2])

        # persistent work tiles
        e = work.tile([128, O, T, B], BF16, name="e")
        rz_f = work.tile([128, T, B], F32, name="rz_f")
        rzb = work.tile([128, T, B], BF16, name="rzb")
        v_f = work.tile([64, O, J], F32, name="v_f")
        v_b = work.tile([64, O, J], BF16, name="v_b")
        v_jb = work.tile([128, 3, B], BF16, name="v_jb")
        s2 = work.tile([64, O, J], F32, name="s2")
        sq = work.tile([64, O], F32, name="sq")
        t1 = work.tile([64, O], F32, name="t1")
        den = work.tile([64, O], F32, name="den")
        rcp = work.tile([64, O], F32, name="rcp")
        ff = work.tile([64, O], F32, name="ff")

        s_ps = psS.tile([64, O, J], F32, name="s_ps")

        def m1_A_chain(h):
            """it0 half h: c uniform -> s_raw[b, o-half] = sum_{i,d} W u."""
            o5 = slice(5 * h, 5 * h + 5)
            for t in range(T):
                for d in range(D):
                    td = t * D + d
                    nc.tensor.matmul(
                        s_ps[:, o5, :].rearrange("p o j -> p (o j)"),
                        ui[:, t, d, :],
                        ws[:, t, d, o5, :].rearrange("p o j -> p (o j)"),
                        start=(td == 0), stop=(td == KC - 1),
                        tile_position=(0, 0), skip_group_check=True,
                    )

        def squash(it, h):
            """v[:, half] = squash(scale * s_ps[:, half]), tiny b-part ops."""
            scale = 0.1 if it == 0 else 1.0
            o5 = slice(5 * h, 5 * h + 5)
            nc.scalar.activation(s2[:, o5, :], s_ps[:, o5, :], ACTF.Square,
                                 scale=scale)
            nc.vector.tensor_reduce(sq[:, o5], s2[:, o5, :], axis=AX.X,
                                    op=ALU.add)
            nc.scalar.activation(t1[:, o5], sq[:, o5], ACTF.Ln, bias=eps1)
            nc.scalar.activation(den[:, o5], t1[:, o5], ACTF.Exp, scale=0.5)
            nc.vector.tensor_scalar_add(t1[:, o5], sq[:, o5], 1.0)
            nc.vector.tensor_tensor(den[:, o5], den[:, o5], t1[:, o5],
                                    op=ALU.mult)
            nc.vector.reciprocal(rcp[:, o5], den[:, o5])
            nc.vector.tensor_tensor(ff[:, o5], sq[:, o5], rcp[:, o5],
                                    op=ALU.mult)
            if it == 0:
                nc.vector.tensor_scalar_mul(ff[:, o5], ff[:, o5], scale)
            nc.vector.tensor_tensor(
                v_f[:, o5, :], s_ps[:, o5, :],
                ff[:, o5].unsqueeze(2).broadcast_to([64, 5, J]),
                op=ALU.mult)
            nc.vector.tensor_copy(
                v_b[:, o5, :].rearrange("p o j -> p (o j)"),
                v_f[:, o5, :].rearrange("p o j -> p (o j)"))

        # half-h (g, sl) slots are disjoint: h0 -> sl0 strips + (g0, sl1);
        # h1 -> (g1..3, sl1) + (g0..1, sl2).
        def transposes(h):
            vt = psVT.tile([128, 3, B], BF16, name="vt", tag="vt0",
                           bufs=1)
            for o in range(5 * h, 5 * h + 5):
                g, sl = o % 4, o // 4
                nc.tensor.matmul(
                    vt[32 * g : 32 * g + 16, sl, :],
                    v_b[:, o, :], id64,
                    is_transpose=True, tile_position=(0, 32 * g),
                )
            for o in range(5 * h, 5 * h + 5):
                g, sl = o % 4, o // 4
                if VJB_ENG == "a":
                    nc.scalar.copy(v_jb[32 * g : 32 * g + 16, sl, :],
                                   vt[32 * g : 32 * g + 16, sl, :])
                else:
                    nc.vector.tensor_copy(
                        v_jb[32 * g : 32 * g + 16, sl, :],
                        vt[32 * g : 32 * g + 16, sl, :])

        flat = lambda ap: ap.rearrange("p t b -> p (t b)")
        flat3 = lambda ap: ap.rearrange("p a b -> p (a b)")

        def emit_G(o, route):
            """G^T chunks for o; returns the ug tile being filled."""
            g, sl = o % 4, o // 4
            ug = ugp.tile([128, KC, B], BF16, name="ug", tag="ug")
            gsb = None
            if route != "b":
                gsb = gsbp.tile([128, KC, B], BF16, name="gsb", tag="gsb")

            def gmm(pg, kk, kc):
                nc.tensor.matmul(
                    pg[:, kk, :],
                    wb2[32 * g : 32 * g + 16, sl, kc, :],
                    v_jb[32 * g : 32 * g + 16, sl, :],
                    start=True, stop=True,
                    tile_position=(32 * g, 0),
                )

            if route == "6":
                # 16-kc psG tiles: (Act, Act, DVE, DVE, Act-half); Pool
                # multiplies the Act-copied parts.
                for ti in range(5):
                    k0 = 16 * ti
                    nk = 16 if ti < 4 else 8
                    pg = psG.tile([128, 16, B], F32, name="pg", tag="pg")
                    for kk in range(nk):
                        gmm(pg, kk, k0 + kk)
                    slk = slice(k0, k0 + nk)
                    if ti in (0, 1, 4):
                        nc.scalar.copy(flat3(gsb[:, slk, :]),
                                       flat3(pg[:, 0:nk, :]))
                    else:
                        nc.vector.tensor_tensor(
                            flat3(ug[:, slk, :]), flat3(pg[:, 0:nk, :]),
                            flat3(ui2[:, slk, :]), op=ALU.mult)
                nc.gpsimd.tensor_tensor(
                    flat3(ug[:, 0:32, :]), flat3(gsb[:, 0:32, :]),
                    flat3(ui2[:, 0:32, :]), op=ALU.mult)
                nc.gpsimd.tensor_tensor(
                    flat3(ug[:, 64:72, :]), flat3(gsb[:, 64:72, :]),
                    flat3(ui2[:, 64:72, :]), op=ALU.mult)
                return ug

            nA = int(M_H[o]) if route in ("m", "M") else 9
            for h in range(9):
                pg = psG.tile([128, 8, B], F32, name="pg", tag="pg")
                for kk in range(8):
                    gmm(pg, kk, 8 * h + kk)
                sl8 = slice(8 * h, 8 * h + 8)
                if route in ("a", "A") or (route in ("m", "M") and h < nA):
                    nc.scalar.copy(flat3(gsb[:, sl8, :]),
                                   flat3(pg[:, 0:8, :]))
                else:  # DVE straight from PSUM
                    nc.vector.tensor_tensor(
                        flat3(ug[:, sl8, :]), flat3(pg[:, 0:8, :]),
                        flat3(ui2[:, sl8, :]), op=ALU.mult)
            if route in ("a", "A"):
                meng = nc.gpsimd if route == "A" else nc.vector
                meng.tensor_tensor(flat3(ug), flat3(gsb), flat3(ui2),
                                   op=ALU.mult)
            elif route in ("m", "M"):
                hA = slice(0, 8 * nA)  # the Act-copied chunk-groups
                meng = nc.gpsimd if route == "M" else nc.vector
                meng.tensor_tensor(
                    flat3(ug[:, hA, :]), flat3(gsb[:, hA, :]),
                    flat3(ui2[:, hA, :]), op=ALU.mult)
            return ug

        def emit_ds(o, ug):
            """delta[o] = sum_d ug chunks.  Returns (psum_tile|None, ug)."""
            eng = DS_ENG[o]
            if eng == "T":  # PE identity-matmul chains into PSUM f32
                ds = psDS.tile([128, T, B], F32, name="ds", tag="ds")
                for d in range(D):
                    nc.tensor.matmul(
                        flat3(ds[:, 0:8, :]), id128,
                        flat3(ug[:, d * T : d * T + 8, :]),
                        start=(d == 0), stop=(d == D - 1),
                        tile_position=(0, 0), skip_group_check=True,
                    )
                for d in range(D):
                    nc.tensor.matmul(
                        ds[:, 8, :], id128, ug[:, d * T + 8, :],
                        start=(d == 0), stop=(d == D - 1),
                        tile_position=(0, 0), skip_group_check=True,
                    )
                return ds, ug
            ve = nc.vector if eng == "v" else nc.gpsimd
            # in-place bf16 fold tree: 72 -> 36 -> 18 -> 9 chunks
            for w in (36, 18, 9):
                ve.tensor_tensor(flat3(ug[:, 0:w, :]), flat3(ug[:, 0:w, :]),
                                 flat3(ug[:, w : 2 * w, :]), op=ALU.add)
            return None, ug

        def emit_exp(o, dsug, r):
            ds, ug = dsug
            src = flat3(ds) if ds is not None else flat3(ug[:, 0:T, :])
            if r == 0:
                nc.scalar.activation(flat(e[:, o]), src, ACTF.Exp)
            else:
                et = etp.tile([128, T, B], BF16, name="et", tag="et")
                nc.scalar.activation(flat(et), src, ACTF.Exp)
                eeng = nc.gpsimd if E_ENG[o] == "g" else nc.vector
                eeng.tensor_tensor(flat(e[:, o]), flat(e[:, o]),
                                   flat(et), op=ALU.mult)

        def emit_zpair(q):
            """partial softmax sums on Pool, overlapped with m2."""
            zq = zp.tile([128, T, B], BF16, name="zq", tag=f"z{q}", bufs=1)
            nc.gpsimd.tensor_tensor(flat(zq), flat(e[:, 2 * q]),
                                    flat(e[:, 2 * q + 1]), op=ALU.add)
            _zpart.append(zq)
            if q in (1, 3):  # fold pairs into quads as soon as available
                zz = zp.tile([128, T, B], BF16, name="zz", tag=f"zz{q}",
                             bufs=1)
                nc.gpsimd.tensor_tensor(flat(zz), flat(_zpart[-2]),
                                        flat(_zpart[-1]), op=ALU.add)
                _zquad.append(zz)

        DS_SHIFT = int(os.environ.get("DS_SHIFT", "3"))

        def m2(r, it):
            """delta_o for all o -> e (pass r), software-pipelined.
            Caller has emitted squash(it,0)+transposes(0); squash/transposes
            of the second half are interleaved after G(1)."""
            ugs = {}
            dss = {}
            for step in range(O + DS_SHIFT + 1):
                if step < O:
                    ugs[step] = emit_G(step, ROUTES[step])
                if step == 1:
                    squash(it, 1)
                    transposes(1)
                if 0 <= step - DS_SHIFT < O:
                    dss[step - DS_SHIFT] = emit_ds(
                        step - DS_SHIFT, ugs.pop(step - DS_SHIFT))
                if 0 <= step - DS_SHIFT - 1 < O:
                    oo = step - DS_SHIFT - 1
                    emit_exp(oo, dss.pop(oo), r)
                    if oo % 2 == 1:
                        emit_zpair(oo // 2)
                if step == 3 and len(_zpart) >= 2:
                    pass

        def softmax_tail():
            """finish Z = sum_o e; rz = 1/Z (bf16)."""
            za = zp.tile([128, T, B], BF16, name="za", tag="za")
            nc.vector.tensor_tensor(flat(za), flat(_zquad[0]),
                                    flat(_zquad[1]), op=ALU.add)
            nc.vector.tensor_tensor(flat(za), flat(za), flat(_zpart[4]),
                                    op=ALU.add)
            with nc.allow_low_precision("softmax normalizer, 2e-2 tolerance"):
                nc.vector.reciprocal(flat(rzb), flat(za))
            _zpart.clear()
            _zquad.clear()

        def m1_B(it):
            """s[b, o, j] = sum_{i,d} (c_o u) W for all o.
            squash/transposes of half 0 are emitted after o=4's chain."""
            for o in range(O):
                co = cop.tile([128, T, B], BF16, name="co", tag="co")
                ceng = nc.gpsimd if C_ENG[(it - 1) * O + o] == "g" else nc.vector
                ceng.tensor_tensor(flat(co), flat(e[:, o]), flat(rzb),
                                   op=ALU.mult)
                cu = cup.tile([128, T, D, B], BF16, name="cu", tag="cu")
                cueng = nc.gpsimd if CU_ENG[(it - 1) * O + o] == "g" else nc.vector
                cueng.tensor_tensor(
                    cu[:, :, :, :],
                    co.unsqueeze(2).broadcast_to([128, T, D, B]),
                    ui[:, :, :, :], op=ALU.mult)
                for t in range(T):
                    for d in range(D):
                        td = t * D + d
                        nc.tensor.matmul(
                            s_ps[:, o, :], cu[:, t, d, :], ws[:, t, d, o, :],
                            start=(td == 0), stop=(td == KC - 1),
                            tile_position=(0, 0), skip_group_check=True,
                        )
                if o == 4:
                    squash(it, 0)
                    if it < 2:
                        transposes(0)
                    else:
                        nc.sync.dma_start(out=vout_d[:, 0:5, :],
                                          in_=v_f[:, 0:5, :])

        _zpart = []
        _zquad = []

        # ========================= flow =========================
        W1 = int(os.environ.get("WARM1", "0"))
        W2 = int(os.environ.get("WARM2", "0"))
        m1_A_chain(0)
        squash(0, 0)
        m1_A_chain(1)
        transposes(0)
        for r in range(2):
            m2(r, r)
            pe_keepalive(W1)
            softmax_tail()
            m1_B(r + 1)
            pe_keepalive(W2)
        squash(2, 1)
        nc.sync.dma_start(out=vout_d[:, 5:10, :], in_=v_f[:, 5:10, :])

    nc.finalize()
    return nc


def _host_prep(u, weights):
    """Per-core input maps. u [512,1152,8] f32, weights [1152,10,16,8] f32."""
    W = np.asarray(weights, dtype=np.float32)
    u = np.asarray(u, dtype=np.float32)
    # ws[p, t, d, o, j] = W[t*128+p, o, j, d]
    ws = np.ascontiguousarray(
        W.reshape(T, 128, O, J, D).transpose(1, 0, 4, 2, 3)
    ).astype(bfnp)
    # wb2[32g+jj, sl, kc, m] = W[c*128+m, o, jj, d], kc = d*T + c
    wt = W.reshape(T, 128, O, J, D)  # [c, m, o, j, d]
    wb2 = np.zeros((128, 3, KC, 128), dtype=bfnp)
    for o in range(O):
        g, sl = o % 4, o // 4
        blk = wt[:, :, o, :, :].transpose(2, 3, 0, 1)  # [j, d, c, m]
        wb2[32 * g : 32 * g + 16, sl] = blk.reshape(J, KC, 128).astype(bfnp)
    id128 = np.eye(128, dtype=np.float32).astype(bfnp)
    id64 = np.eye(64, dtype=np.float32).astype(bfnp)

    base = {"ws": ws, "wb2": wb2, "id128": id128, "id64": id64}
    in_maps = []
    for c in range(N_CORES):
        uc = u[c * B : (c + 1) * B]  # [64, 1152, 8]
        ur = uc.reshape(B, T, 128, D)
        ui = np.ascontiguousarray(ur.transpose(2, 1, 3, 0)).astype(bfnp)
        # ui2[p, kc, b] = u[b, c*128+p, d], kc = d*T + c
        ui2 = np.ascontiguousarray(
            ur.transpose(2, 3, 1, 0).reshape(128, D * T, B)
        ).astype(bfnp)
        in_maps.append({**base, "ui": ui, "ui2": ui2})
    return in_maps


def kernel(u, weights):
    if "nc" not in _cache:
        _cache["nc"] = build_nc()
    nc = _cache["nc"]
    in_maps = _host_prep(u, weights)
    res = run_bass_kernel_spmd(nc, in_maps, core_ids=list(range(N_CORES)))
    out = np.concatenate([res.results[c]["vout"] for c in range(N_CORES)], axis=0)
    return out.astype(np.float32)


if __name__ == "__main__":
    rng = np.random.default_rng(0)
    u = rng.standard_normal((512, 1152, 8), dtype=np.float32)
    w = (rng.standard_normal((1152, 10, 16, 8)) * 0.1).astype(np.float32)
    v = kernel(u, w)
    print("out", v.shape, v.dtype, np.abs(v).max())


# revision 6
# speedup vs baseline: 1.0541x; 1.0044x over previous
"""CapsNet dynamic-routing layer on 8 Trainium2 NeuronCores (Bass/Tile), v2.

reference math (per batch element b):
  u_hat[b,i,o,j] = sum_d W[i,o,j,d] * u[b,i,d]        (never materialized)
  bl = 0; for r in 0..2:
    c = softmax_o(bl); s[b,o,j] = sum_i c*u_hat; v = squash(s)
    if r < 2: bl += sum_j u_hat*v
  return v  [B, 10, 16]

Distribution: pure data parallel, batch 512 -> 64 per core x 8 cores;
weights replicated.  Per-core: b=64, i=1152=9*128, o=10, j=16, d=8.

v2 design vs v1:
  * m1 (s-matmuls) output b-partitioned [64, (o,j)] -> n=16/o per
    instruction instead of n=64: 4x fewer PE rows.
  * m2 (agreement) G^T = W^T v computed (d,i)-partitioned: out
    [128=(d,i)chunk, 64 b] per kc-chunk: 2x fewer PE rows than v1.
  * d-reduction done ON PE via chained identity matmuls accumulating
    in PSUM (start/stop over d) - removes the DVE add tree entirely.
  * logits bl never materialized: e2 = e1 * exp(delta2) folds the
    cross-iteration accumulation into the exp chain.
  * softmax stays i-partitioned end-to-end: no DMA transposes.
  * squash in b-partitioned layout: a handful of [64, 10]-sized ops.
  * PSUM->SBUF conversion work (f32 G -> bf16 for 2x-mode DVE mult)
    is routed per-o across Act / DVE-direct / Pool to balance engines.
"""
import sys

sys.path.insert(0, "/opt/trn_rl_repo")

import numpy as np
import ml_dtypes
from contextlib import ExitStack

from concourse import bacc, mybir, hw_specs
from concourse.tile import TileContext
from concourse.bass_utils import run_bass_kernel_spmd

BF16 = mybir.dt.bfloat16
F32 = mybir.dt.float32
AX = mybir.AxisListType
ALU = mybir.AluOpType
ACTF = mybir.ActivationFunctionType
bfnp = ml_dtypes.bfloat16

B = 64
I = 1152
T = 9          # i-chunks of 128 (also the softmax "c" index)
O = 10
J = 16
D = 8
KC = T * D     # 72 k-chunks of 128 over flat k = d*I + i
EPS = 1e-06
N_CORES = 8

_cache = {}

# Route every activation through the one table set that has exp+ln+copy,
# so the ACT engine never reloads tables mid-kernel.
_KEEP_SET = "natural_log_exp_and_others"


def _patched_tables(arch):
    full = {k: set(v) for k, v in hw_specs.get_activation_tables(arch).items()}
    keep = full[_KEEP_SET]
    return {k: (v if k == _KEEP_SET else v - keep) for k, v in full.items()}


import os
if os.environ.get('ACT_PATCH', '1') == '1':
    bacc.get_activation_tables = _patched_tables

# per-o conversion route for the m2 ug-mult (GPSIMD cannot read PSUM,
# so conversions are Act or DVE only):
#   'a' = Act copies PSUM f32 -> SBUF bf16, DVE multiplies at 2x
#   'A' = Act copies, Pool multiplies (SBUF-only, legal)
#   'b' = DVE multiplies straight from PSUM f32 (1x, no Act work)
#   'm' = per-chunk alternation: even chunks Act-copy, odd chunks
#         DVE-direct; DVE strided 2x mult for the copied half
ROUTES = os.environ.get("M2_ROUTES", "MMMMMMMMMM")
# per-o d-sum engine: 'T' PE identity-matmul chain (psum f32)
#                     'v' DVE in-place bf16 fold tree | 'g' Pool fold tree
DS_ENG = os.environ.get("DS_ENG", "TgTTvTTgvT")
# per-(it,o) cu engine: 'v' DVE | 'g' Pool  (20 chars: it1 o0-9, it2 o0-9)
CU_ENG = os.environ.get("CU_ENG", "vgvvgvgvgv" "vgvvgvgvgv")
# engine for per-o c = e*rz mult: 'v' DVE | 'g' Pool
C_ENG = os.environ.get("C_ENG", "gv" * 10)
if len(C_ENG) == 1:
    C_ENG = C_ENG * 20
# per-o count of Act-copied chunk-groups in m/M routes (rest DVE-direct)
M_H = os.environ.get("M_H", "5555545545")
# engine for v_jb strip copies: 'a' Act | 'v' DVE
VJB_ENG = os.environ.get("VJB_ENG", "v")
# per-o engine for the pass-2 e = e*exp(delta) mult: 'v' DVE | 'g' Pool
E_ENG = os.environ.get("E_ENG", "vgvgvgvgvg")


def build_nc():
    nc = bacc.Bacc()
    ws_d = nc.dram_tensor("ws", [128, T, D, O, J], BF16, kind="ExternalInput")
    ui_d = nc.dram_tensor("ui", [128, T, D, B], BF16, kind="ExternalInput")
    ui2_d = nc.dram_tensor("ui2", [128, KC, B], BF16, kind="ExternalInput")
    wb2_d = nc.dram_tensor("wb2", [128, 3, KC, 128], BF16, kind="ExternalInput")
    id128_d = nc.dram_tensor("id128", [128, 128], BF16, kind="ExternalInput")
    id64_d = nc.dram_tensor("id64", [64, 64], BF16, kind="ExternalInput")
    vout_d = nc.dram_tensor("vout", [B, O, J], F32, kind="ExternalOutput")

    with TileContext(nc) as tc, ExitStack() as ctx:
        static = ctx.enter_context(tc.tile_pool(name="static", bufs=1))
        work = ctx.enter_context(tc.tile_pool(name="work", bufs=1))
        gsbp = ctx.enter_context(tc.tile_pool(name="gsbp", bufs=2))
        ugp = ctx.enter_context(tc.tile_pool(
            name="ugp", bufs=int(os.environ.get("DS_SHIFT", "3")) + 2))
        cup = ctx.enter_context(tc.tile_pool(name="cup", bufs=2))
        etp = ctx.enter_context(tc.tile_pool(name="etp", bufs=2))
        cop = ctx.enter_context(tc.tile_pool(name="cop", bufs=2))
        zp = ctx.enter_context(tc.tile_pool(name="zp", bufs=2))
        psS = ctx.enter_context(tc.tile_pool(name="psS", bufs=1, space="PSUM"))
        psVT = ctx.enter_context(tc.tile_pool(name="psVT", bufs=1, space="PSUM"))
        psG = ctx.enter_context(tc.tile_pool(
            name="psG", bufs=int(os.environ.get("PSG_BUFS", "4")), space="PSUM"))
        psDS = ctx.enter_context(tc.tile_pool(
            name="psDS", bufs=int(os.environ.get("PSDS_BUFS", "1")), space="PSUM"))

        # PE p-state: the clock ramps per continuous-busy stretch (reset on
        # idle; full speed only after 3us busy).  Dummy matmuls keep the PE
        # clock hot through DMA waits and phase boundaries.
        warm = static.tile([128, 128], BF16, name="warm")
        nc.vector.memset(warm, 0.0)

        def pe_keepalive(n):
            for _ in range(n):
                wps = psVT.tile([128, 96], F32, name="wps", tag="vt0")
                nc.tensor.matmul(wps, warm, warm[:, 0:96], start=True,
                                 stop=True, tile_position=(0, 0))

        pe_keepalive(int(os.environ.get("WARM0", "75")))

        ws = static.tile([128, T, D, O, J], BF16, name="ws")
        ui = static.tile([128, T, D, B], BF16, name="ui")
        ui2 = static.tile([128, KC, B], BF16, name="ui2")
        wb2 = static.tile([128, 3, KC, 128], BF16, name="wb2")
        id128 = static.tile([128, 128], BF16, name="id128")
        id64 = static.tile([64, 64], BF16, name="id64")
        eps1 = static.tile([64, 1], F32, name="eps1")
        nc.vector.memset(eps1, EPS)

        # DMA cost model (legacy CoreSim): each DMA holds the issuing
        # engine's queue for ~1.7us fixed + per-partition-free-bytes *
        # 0.39ns.  So: few big DMAs, spread across the SP / Act / Pool
        # queues, ordered by first use.
        # SP:   ws t0-4, id64, wb2 slot1, wb2 slot2
        # Pool: ws t5-8, wb2 slot0, id128
        # Act:  ui, ui2   (Act must be free for squash-0 at ~12us)
        if os.environ.get("DMA_PLAN", "A") == "A":
            nc.sync.dma_start(out=ws[:, 0:5], in_=ws_d[:, 0:5])
            nc.gpsimd.dma_start(out=ws[:, 5:9], in_=ws_d[:, 5:9])
            nc.scalar.dma_start(out=ui, in_=ui_d[:, :])
            nc.sync.dma_start(out=id64, in_=id64_d[:, :])
            nc.gpsimd.dma_start(out=wb2[:, 0], in_=wb2_d[:, 0])
            nc.gpsimd.dma_start(out=id128, in_=id128_d[:, :])
            nc.scalar.dma_start(out=ui2, in_=ui2_d[:, :])
            nc.sync.dma_start(out=wb2[:, 1], in_=wb2_d[:, 1])
            nc.sync.dma_start(out=wb2[:, 2], in_=wb2_d[:, 2])
        else:
            # plan E: per-t ws chunks alternating SP/Pool so the m1_A chain
            # streams at DMA pitch; slot0 split across both queues after.
            for t in range(0, 9, 2):
                nc.sync.dma_start(out=ws[:, t], in_=ws_d[:, t])
            for t in range(1, 9, 2):
                nc.gpsimd.dma_start(out=ws[:, t], in_=ws_d[:, t])
            nc.scalar.dma_start(out=ui, in_=ui_d[:, :])
            nc.scalar.dma_start(out=id64, in_=id64_d[:, :])
            nc.sync.dma_start(out=wb2[:, 0, 0:36], in_=wb2_d[:, 0, 0:36])
            nc.gpsimd.dma_start(out=wb2[:, 0, 36:72], in_=wb2_d[:, 0, 36:72])
            nc.scalar.dma_start(out=ui2, in_=ui2_d[:, :])
            nc.sync.dma_start(out=id128, in_=id128_d[:, :])
            nc.sync.dma_start(out=wb2[:, 1], in_=wb2_d[:, 1])
            nc.gpsimd.dma_start(out=wb2[:, 2], in_=wb2_d[:, 2])

        # persistent work tiles
        e = work.tile([128, O, T, B], BF16, name="e")
        rz_f = work.tile([128, T, B], F32, name="rz_f")
        rzb = work.tile([128, T, B], BF16, name="rzb")
        v_f = work.tile([64, O, J], F32, name="v_f")
        v_b = work.tile([64, O, J], BF16, name="v_b")
        v_jb = work.tile([128, 3, B], BF16, name="v_jb")
        s2 = work.tile([64, O, J], F32, name="s2")
        sq = work.tile([64, O], F32, name="sq")
        t1 = work.tile([64, O], F32, name="t1")
        den = work.tile([64, O], F32, name="den")
        rcp = work.tile([64, O], F32, name="rcp")
        ff = work.tile([64, O], F32, name="ff")

        s_ps = psS.tile([64, O, J], F32, name="s_ps")

        def m1_A_chain(h):
            """it0 half h: c uniform -> s_raw[b, o-half] = sum_{i,d} W u."""
            o5 = slice(5 * h, 5 * h + 5)
            for t in range(T):
                for d in range(D):
                    td = t * D + d
                    nc.tensor.matmul(
                        s_ps[:, o5, :].rearrange("p o j -> p (o j)"),
                        ui[:, t, d, :],
                        ws[:, t, d, o5, :].rearrange("p o j -> p (o j)"),
                        start=(td == 0), stop=(td == KC - 1),
                        tile_position=(0, 0), skip_group_check=True,
                    )

        def squash(it, h):
            """v[:, half] = squash(scale * s_ps[:, half]), tiny b-part ops."""
            scale = 0.1 if it == 0 else 1.0
            o5 = slice(5 * h, 5 * h + 5)
            nc.scalar.activation(s2[:, o5, :], s_ps[:, o5, :], ACTF.Square,
                                 scale=scale)
            nc.vector.tensor_reduce(sq[:, o5], s2[:, o5, :], axis=AX.X,
                                    op=ALU.add)
            nc.scalar.activation(t1[:, o5], sq[:, o5], ACTF.Ln, bias=eps1)
            nc.scalar.activation(den[:, o5], t1[:, o5], ACTF.Exp, scale=0.5)
            nc.vector.tensor_scalar_add(t1[:, o5], sq[:, o5], 1.0)
            nc.vector.tensor_tensor(den[:, o5], den[:, o5], t1[:, o5],
                                    op=ALU.mult)
            nc.vector.reciprocal(rcp[:, o5], den[:, o5])
            nc.vector.tensor_tensor(ff[:, o5], sq[:, o5], rcp[:, o5],
                                    op=ALU.mult)
            if it == 0:
                nc.vector.tensor_scalar_mul(ff[:, o5], ff[:, o5], scale)
            nc.vector.tensor_tensor(
                v_f[:, o5, :], s_ps[:, o5, :],
                ff[:, o5].unsqueeze(2).broadcast_to([64, 5, J]),
                op=ALU.mult)
            nc.vector.tensor_copy(
                v_b[:, o5, :].rearrange("p o j -> p (o j)"),
                v_f[:, o5, :].rearrange("p o j -> p (o j)"))

        # half-h (g, sl) slots are disjoint: h0 -> sl0 strips + (g0, sl1);
        # h1 -> (g1..3, sl1) + (g0..1, sl2).
        def transposes(h):
            vt = psVT.tile([128, 3, B], BF16, name="vt", tag="vt0",
                           bufs=1)
            for o in range(5 * h, 5 * h + 5):
                g, sl = o % 4, o // 4
                nc.tensor.matmul(
                    vt[32 * g : 32 * g + 16, sl, :],
                    v_b[:, o, :], id64,
                    is_transpose=True, tile_position=(0, 32 * g),
                )
            for o in range(5 * h, 5 * h + 5):
                g, sl = o % 4, o // 4
                if VJB_ENG == "a":
                    nc.scalar.copy(v_jb[32 * g : 32 * g + 16, sl, :],
                                   vt[32 * g : 32 * g + 16, sl, :])
                else:
                    nc.vector.tensor_copy(
                        v_jb[32 * g : 32 * g + 16, sl, :],
                        vt[32 * g : 32 * g + 16, sl, :])

        flat = lambda ap: ap.rearrange("p t b -> p (t b)")
        flat3 = lambda ap: ap.rearrange("p a b -> p (a b)")

        def emit_G(o, route):
            """G^T chunks for o; returns the ug tile being filled."""
            g, sl = o % 4, o // 4
            ug = ugp.tile([128, KC, B], BF16, name="ug", tag="ug")
            gsb = None
            if route != "b":
                gsb = gsbp.tile([128, KC, B], BF16, name="gsb", tag="gsb")

            def gmm(pg, kk, kc):
                nc.tensor.matmul(
                    pg[:, kk, :],
                    wb2[32 * g : 32 * g + 16, sl, kc, :],
                    v_jb[32 * g : 32 * g + 16, sl, :],
                    start=True, stop=True,
                    tile_position=(32 * g, 0),
                )

            if route == "6":
                # 16-kc psG tiles: (Act, Act, DVE, DVE, Act-half); Pool
                # multiplies the Act-copied parts.
                for ti in range(5):
                    k0 = 16 * ti
                    nk = 16 if ti < 4 else 8
                    pg = psG.tile([128, 16, B], F32, name="pg", tag="pg")
                    for kk in range(nk):
                        gmm(pg, kk, k0 + kk)
                    slk = slice(k0, k0 + nk)
                    if ti in (0, 1, 4):
                        nc.scalar.copy(flat3(gsb[:, slk, :]),
                                       flat3(pg[:, 0:nk, :]))
                    else:
                        nc.vector.tensor_tensor(
                            flat3(ug[:, slk, :]), flat3(pg[:, 0:nk, :]),
                            flat3(ui2[:, slk, :]), op=ALU.mult)
                nc.gpsimd.tensor_tensor(
                    flat3(ug[:, 0:32, :]), flat3(gsb[:, 0:32, :]),
                    flat3(ui2[:, 0:32, :]), op=ALU.mult)
                nc.gpsimd.tensor_tensor(
                    flat3(ug[:, 64:72, :]), flat3(gsb[:, 64:72, :]),
                    flat3(ui2[:, 64:72, :]), op=ALU.mult)
                return ug

            nA = int(M_H[o]) if route in ("m", "M") else 9
            for h in range(9):
                pg = psG.tile([128, 8, B], F32, name="pg", tag="pg")
                for kk in range(8):
                    gmm(pg, kk, 8 * h + kk)
                sl8 = slice(8 * h, 8 * h + 8)
                if route in ("a", "A") or (route in ("m", "M") and h < nA):
                    nc.scalar.copy(flat3(gsb[:, sl8, :]),
                                   flat3(pg[:, 0:8, :]))
                else:  # DVE straight from PSUM
                    nc.vector.tensor_tensor(
                        flat3(ug[:, sl8, :]), flat3(pg[:, 0:8, :]),
                        flat3(ui2[:, sl8, :]), op=ALU.mult)
            if route in ("a", "A"):
                meng = nc.gpsimd if route == "A" else nc.vector
                meng.tensor_tensor(flat3(ug), flat3(gsb), flat3(ui2),
                                   op=ALU.mult)
            elif route in ("m", "M"):
                hA = slice(0, 8 * nA)  # the Act-copied chunk-groups
                meng = nc.gpsimd if route == "M" else nc.vector
                meng.tensor_tensor(
                    flat3(ug[:, hA, :]), flat3(gsb[:, hA, :]),
                    flat3(ui2[:, hA, :]), op=ALU.mult)
            return ug

        def emit_ds(o, ug):
            """delta[o] = sum_d ug chunks.  Returns (psum_tile|None, ug)."""
            eng = DS_ENG[o]
            if eng == "T":  # PE identity-matmul chains into PSUM f32
                ds = psDS.tile([128, T, B], F32, name="ds", tag="ds")
                for d in range(D):
                    nc.tensor.matmul(
                        flat3(ds[:, 0:8, :]), id128,
                        flat3(ug[:, d * T : d * T + 8, :]),
                        start=(d == 0), stop=(d == D - 1),
                        tile_position=(0, 0), skip_group_check=True,
                    )
                for d in range(D):
                    nc.tensor.matmul(
                        ds[:, 8, :], id128, ug[:, d * T + 8, :],
                        start=(d == 0), stop=(d == D - 1),
                        tile_position=(0, 0), skip_group_check=True,
                    )
                return ds, ug
            ve = nc.vector if eng == "v" else nc.gpsimd
            # in-place bf16 fold tree: 72 -> 36 -> 18 -> 9 chunks
            for w in (36, 18, 9):
                ve.tensor_tensor(flat3(ug[:, 0:w, :]), flat3(ug[:, 0:w, :]),
                                 flat3(ug[:, w : 2 * w, :]), op=ALU.add)
            return None, ug

        def emit_exp(o, dsug, r):
            ds, ug = dsug
            src = flat3(ds) if ds is not None else flat3(ug[:, 0:T, :])
            if r == 0:
                nc.scalar.activation(flat(e[:, o]), src, ACTF.Exp)
            else:
                et = etp.tile([128, T, B], BF16, name="et", tag="et")
                nc.scalar.activation(flat(et), src, ACTF.Exp)
                eeng = nc.gpsimd if E_ENG[o] == "g" else nc.vector
                eeng.tensor_tensor(flat(e[:, o]), flat(e[:, o]),
                                   flat(et), op=ALU.mult)

        def emit_zpair(q):
            """partial softmax sums on Pool, overlapped with m2."""
            zq = zp.tile([128, T, B], BF16, name="zq", tag=f"z{q}", bufs=1)
            nc.gpsimd.tensor_tensor(flat(zq), flat(e[:, 2 * q]),
                                    flat(e[:, 2 * q + 1]), op=ALU.add)
            _zpart.append(zq)
            if q in (1, 3):  # fold pairs into quads as soon as available
                zz = zp.tile([128, T, B], BF16, name="zz", tag=f"zz{q}",
                             bufs=1)
                nc.gpsimd.tensor_tensor(flat(zz), flat(_zpart[-2]),
                                        flat(_zpart[-1]), op=ALU.add)
                _zquad.append(zz)

        DS_SHIFT = int(os.environ.get("DS_SHIFT", "3"))

        def m2(r, it):
            """delta_o for all o -> e (pass r), software-pipelined.
            Caller has emitted squash(it,0)+transposes(0); squash/transposes
            of the second half are interleaved after G(1)."""
            ugs = {}
            dss = {}
            for step in range(O + DS_SHIFT + 1):
                if step < O:
                    ugs[step] = emit_G(step, ROUTES[step])
                if step == 1:
                    squash(it, 1)
                    transposes(1)
                if 0 <= step - DS_SHIFT < O:
                    dss[step - DS_SHIFT] = emit_ds(
                        step - DS_SHIFT, ugs.pop(step - DS_SHIFT))
                if 0 <= step - DS_SHIFT - 1 < O:
                    oo = step - DS_SHIFT - 1
                    emit_exp(oo, dss.pop(oo), r)
                    if oo % 2 == 1:
                        emit_zpair(oo // 2)
                if step == 3 and len(_zpart) >= 2:
                    pass

        def softmax_tail():
            """finish Z = sum_o e; rz = 1/Z (bf16)."""
            za = zp.tile([128, T, B], BF16, name="za", tag="za")
            nc.vector.tensor_tensor(flat(za), flat(_zquad[0]),
                                    flat(_zquad[1]), op=ALU.add)
            nc.vector.tensor_tensor(flat(za), flat(za), flat(_zpart[4]),
                                    op=ALU.add)
            with nc.allow_low_precision("softmax normalizer, 2e-2 tolerance"):
                nc.vector.reciprocal(flat(rzb), flat(za))
            _zpart.clear()
            _zquad.clear()

        def m1_B(it):
            """s[b, o, j] = sum_{i,d} (c_o u) W for all o.
            squash/transposes of half 0 are emitted after o=4's chain."""
            for o in range(O):
                co = cop.tile([128, T, B], BF16, name="co", tag="co")
                ceng = nc.gpsimd if C_ENG[(it - 1) * O + o] == "g" else nc.vector
                ceng.tensor_tensor(flat(co), flat(e[:, o]), flat(rzb),
                                   op=ALU.mult)
                cu = cup.tile([128, T, D, B], BF16, name="cu", tag="cu")
                cueng = nc.gpsimd if CU_ENG[(it - 1) * O + o] == "g" else nc.vector
                cueng.tensor_tensor(
                    cu[:, :, :, :],
                    co.unsqueeze(2).broadcast_to([128, T, D, B]),
                    ui[:, :, :, :], op=ALU.mult)
                for t in range(T):
                    for d in range(D):
                        td = t * D + d
                        nc.tensor.matmul(
                            s_ps[:, o, :], cu[:, t, d, :], ws[:, t, d, o, :],
                            start=(td == 0), stop=(td == KC - 1),
                            tile_position=(0, 0), skip_group_check=True,
                        )
                if o == 4:
                    squash(it, 0)
                    if it < 2:
                        transposes(0)
                    else:
                        nc.sync.dma_start(out=vout_d[:, 0:5, :],
                                          in_=v_f[:, 0:5, :])

        _zpart = []
        _zquad = []

        # ========================= flow =========================
        W1 = int(os.environ.get("WARM1", "0"))
        W2 = int(os.environ.get("WARM2", "0"))
        m1_A_chain(0)
        squash(0, 0)
        m1_A_chain(1)
        transposes(0)
        for r in range(2):
            m2(r, r)
            pe_keepalive(W1)
            softmax_tail()
            m1_B(r + 1)
            pe_keepalive(W2)
        squash(2, 1)
        nc.sync.dma_start(out=vout_d[:, 5:10, :], in_=v_f[:, 5:10, :])

    nc.finalize()
    return nc


def _host_prep(u, weights):
    """Per-core input maps. u [512,1152,8] f32, weights [1152,10,16,8] f32."""
    W = np.asarray(weights, dtype=np.float32)
    u = np.asarray(u, dtype=np.float32)
    # ws[p, t, d, o, j] = W[t*128+p, o, j, d]
    ws = np.ascontiguousarray(
        W.reshape(T, 128, O, J, D).transpose(1, 0, 4, 2, 3)
    ).astype(bfnp)
    # wb2[32g+jj, sl, kc, m] = W[c*128+m, o, jj, d], kc = d*T + c
    wt = W.reshape(T, 128, O, J, D)  # [c, m, o, j, d]
    wb2 = np.zeros((128, 3, KC, 128), dtype=bfnp)
    for o in range(O):
        g, sl = o % 4, o // 4
        blk = wt[:, :, o, :, :].transpose(2, 3, 0, 1)  # [j, d, c, m]
        wb2[32 * g : 32 * g + 16, sl] = blk.reshape(J, KC, 128).astype(bfnp)
    id128 = np.eye(128, dtype=np.float32).astype(bfnp)
    id64 = np.eye(64, dtype=np.float32).astype(bfnp)

    base = {"ws": ws, "wb2": wb2, "id128": id128, "id64": id64}
    in_maps = []
    for c in range(N_CORES):
        uc = u[c * B : (c + 1) * B]  # [64, 1152, 8]
        ur = uc.reshape(B, T, 128, D)
        ui = np.ascontiguousarray(ur.transpose(2, 1, 3, 0)).astype(bfnp)
        # ui2[p, kc, b] = u[b, c*128+p, d], kc = d*T + c
        ui2 = np.ascontiguousarray(
            ur.transpose(2, 3, 1, 0).reshape(128, D * T, B)
        ).astype(bfnp)
        in_maps.append({**base, "ui": ui, "ui2": ui2})
    return in_maps


def kernel(u, weights):
    if "nc" not in _cache:
        _cache["nc"] = build_nc()
    nc = _cache["nc"]
    in_maps = _host_prep(u, weights)
    res = run_bass_kernel_spmd(nc, in_maps, core_ids=list(range(N_CORES)))
    out = np.concatenate([res.results[c]["vout"] for c in range(N_CORES)], axis=0)
    return out.astype(np.float32)


if __name__ == "__main__":
    rng = np.random.default_rng(0)
    u = rng.standard_normal((512, 1152, 8), dtype=np.float32)
    w = (rng.standard_normal((1152, 10, 16, 8)) * 0.1).astype(np.float32)
    v = kernel(u, w)
    print("out", v.shape, v.dtype, np.abs(v).max())


# revision 7
# speedup vs baseline: 1.0617x; 1.0072x over previous
"""CapsNet dynamic-routing layer on 8 Trainium2 NeuronCores (Bass/Tile), v2.

reference math (per batch element b):
  u_hat[b,i,o,j] = sum_d W[i,o,j,d] * u[b,i,d]        (never materialized)
  bl = 0; for r in 0..2:
    c = softmax_o(bl); s[b,o,j] = sum_i c*u_hat; v = squash(s)
    if r < 2: bl += sum_j u_hat*v
  return v  [B, 10, 16]

Distribution: pure data parallel, batch 512 -> 64 per core x 8 cores;
weights replicated.  Per-core: b=64, i=1152=9*128, o=10, j=16, d=8.

v2 design vs v1:
  * m1 (s-matmuls) output b-partitioned [64, (o,j)] -> n=16/o per
    instruction instead of n=64: 4x fewer PE rows.
  * m2 (agreement) G^T = W^T v computed (d,i)-partitioned: out
    [128=(d,i)chunk, 64 b] per kc-chunk: 2x fewer PE rows than v1.
  * d-reduction done ON PE via chained identity matmuls accumulating
    in PSUM (start/stop over d) - removes the DVE add tree entirely.
  * logits bl never materialized: e2 = e1 * exp(delta2) folds the
    cross-iteration accumulation into the exp chain.
  * softmax stays i-partitioned end-to-end: no DMA transposes.
  * squash in b-partitioned layout: a handful of [64, 10]-sized ops.
  * PSUM->SBUF conversion work (f32 G -> bf16 for 2x-mode DVE mult)
    is routed per-o across Act / DVE-direct / Pool to balance engines.
"""
import sys

sys.path.insert(0, "/opt/trn_rl_repo")

import numpy as np
import ml_dtypes
from contextlib import ExitStack

from concourse import bacc, mybir, hw_specs
from concourse.tile import TileContext
from concourse.bass_utils import run_bass_kernel_spmd

BF16 = mybir.dt.bfloat16
F32 = mybir.dt.float32
AX = mybir.AxisListType
ALU = mybir.AluOpType
ACTF = mybir.ActivationFunctionType
bfnp = ml_dtypes.bfloat16

B = 64
I = 1152
T = 9          # i-chunks of 128 (also the softmax "c" index)
O = 10
J = 16
D = 8
KC = T * D     # 72 k-chunks of 128 over flat k = d*I + i
EPS = 1e-06
N_CORES = 8

_cache = {}

# Route every activation through the one table set that has exp+ln+copy,
# so the ACT engine never reloads tables mid-kernel.
_KEEP_SET = "natural_log_exp_and_others"


def _patched_tables(arch):
    full = {k: set(v) for k, v in hw_specs.get_activation_tables(arch).items()}
    keep = full[_KEEP_SET]
    return {k: (v if k == _KEEP_SET else v - keep) for k, v in full.items()}


import os
if os.environ.get('ACT_PATCH', '1') == '1':
    bacc.get_activation_tables = _patched_tables

# per-o conversion route for the m2 ug-mult (GPSIMD cannot read PSUM,
# so conversions are Act or DVE only):
#   'a' = Act copies PSUM f32 -> SBUF bf16, DVE multiplies at 2x
#   'A' = Act copies, Pool multiplies (SBUF-only, legal)
#   'b' = DVE multiplies straight from PSUM f32 (1x, no Act work)
#   'm' = per-chunk alternation: even chunks Act-copy, odd chunks
#         DVE-direct; DVE strided 2x mult for the copied half
ROUTES = os.environ.get("M2_ROUTES", "MMMMMMMMMM")
# per-o d-sum engine: 'T' PE identity-matmul chain (psum f32)
#                     'v' DVE in-place bf16 fold tree | 'g' Pool fold tree
DS_ENG = os.environ.get("DS_ENG", "TgTTvTTgvT")
# per-(it,o) cu engine: 'v' DVE | 'g' Pool  (20 chars: it1 o0-9, it2 o0-9)
CU_ENG = os.environ.get("CU_ENG", "vgvvgvgvgv" "vgvvgvgvgv")
# engine for per-o c = e*rz mult: 'v' DVE | 'g' Pool
C_ENG = os.environ.get("C_ENG", "gv" * 10)
if len(C_ENG) == 1:
    C_ENG = C_ENG * 20
# per-o count of Act-copied chunk-groups in m/M routes (rest DVE-direct)
M_H = os.environ.get("M_H", "5555545545")
# engine for v_jb strip copies: 'a' Act | 'v' DVE
VJB_ENG = os.environ.get("VJB_ENG", "v")
# per-o engine for the pass-2 e = e*exp(delta) mult: 'v' DVE | 'g' Pool
E_ENG = os.environ.get("E_ENG", "vgvgvgvgvg")
SPLIT_MULT = os.environ.get("SPLIT_MULT", "1") == "1"


def build_nc():
    nc = bacc.Bacc()
    ws_d = nc.dram_tensor("ws", [128, T, D, O, J], BF16, kind="ExternalInput")
    ui_d = nc.dram_tensor("ui", [128, T, D, B], BF16, kind="ExternalInput")
    ui2_d = nc.dram_tensor("ui2", [128, KC, B], BF16, kind="ExternalInput")
    wb2_d = nc.dram_tensor("wb2", [128, 3, KC, 128], BF16, kind="ExternalInput")
    id128_d = nc.dram_tensor("id128", [128, 128], BF16, kind="ExternalInput")
    id64_d = nc.dram_tensor("id64", [64, 64], BF16, kind="ExternalInput")
    vout_d = nc.dram_tensor("vout", [B, O, J], F32, kind="ExternalOutput")

    with TileContext(nc) as tc, ExitStack() as ctx:
        static = ctx.enter_context(tc.tile_pool(name="static", bufs=1))
        work = ctx.enter_context(tc.tile_pool(name="work", bufs=1))
        gsbp = ctx.enter_context(tc.tile_pool(name="gsbp", bufs=2))
        ugp = ctx.enter_context(tc.tile_pool(
            name="ugp", bufs=int(os.environ.get("DS_SHIFT", "3")) + 2))
        cup = ctx.enter_context(tc.tile_pool(name="cup", bufs=2))
        etp = ctx.enter_context(tc.tile_pool(name="etp", bufs=2))
        cop = ctx.enter_context(tc.tile_pool(name="cop", bufs=2))
        zp = ctx.enter_context(tc.tile_pool(name="zp", bufs=2))
        psS = ctx.enter_context(tc.tile_pool(name="psS", bufs=1, space="PSUM"))
        psVT = ctx.enter_context(tc.tile_pool(name="psVT", bufs=1, space="PSUM"))
        psG = ctx.enter_context(tc.tile_pool(
            name="psG", bufs=int(os.environ.get("PSG_BUFS", "4")), space="PSUM"))
        psDS = ctx.enter_context(tc.tile_pool(
            name="psDS", bufs=int(os.environ.get("PSDS_BUFS", "1")), space="PSUM"))

        # PE p-state: the clock ramps per continuous-busy stretch (reset on
        # idle; full speed only after 3us busy).  Dummy matmuls keep the PE
        # clock hot through DMA waits and phase boundaries.
        warm = static.tile([128, 128], BF16, name="warm")
        nc.vector.memset(warm, 0.0)

        def pe_keepalive(n):
            for _ in range(n):
                wps = psVT.tile([128, 96], F32, name="wps", tag="vt0")
                nc.tensor.matmul(wps, warm, warm[:, 0:96], start=True,
                                 stop=True, tile_position=(0, 0))

        pe_keepalive(int(os.environ.get("WARM0", "75")))

        ws = static.tile([128, T, D, O, J], BF16, name="ws")
        ui = static.tile([128, T, D, B], BF16, name="ui")
        ui2 = static.tile([128, KC, B], BF16, name="ui2")
        wb2 = static.tile([128, 3, KC, 128], BF16, name="wb2")
        id128 = static.tile([128, 128], BF16, name="id128")
        id64 = static.tile([64, 64], BF16, name="id64")
        eps1 = static.tile([64, 1], F32, name="eps1")
        nc.vector.memset(eps1, EPS)

        # DMA cost model (legacy CoreSim): each DMA holds the issuing
        # engine's queue for ~1.7us fixed + per-partition-free-bytes *
        # 0.39ns.  So: few big DMAs, spread across the SP / Act / Pool
        # queues, ordered by first use.
        # SP:   ws t0-4, id64, wb2 slot1, wb2 slot2
        # Pool: ws t5-8, wb2 slot0, id128
        # Act:  ui, ui2   (Act must be free for squash-0 at ~12us)
        if os.environ.get("DMA_PLAN", "A") == "A":
            nc.sync.dma_start(out=ws[:, 0:5], in_=ws_d[:, 0:5])
            nc.gpsimd.dma_start(out=ws[:, 5:9], in_=ws_d[:, 5:9])
            nc.scalar.dma_start(out=ui, in_=ui_d[:, :])
            nc.sync.dma_start(out=id64, in_=id64_d[:, :])
            nc.gpsimd.dma_start(out=wb2[:, 0], in_=wb2_d[:, 0])
            nc.gpsimd.dma_start(out=id128, in_=id128_d[:, :])
            nc.scalar.dma_start(out=ui2, in_=ui2_d[:, :])
            nc.sync.dma_start(out=wb2[:, 1], in_=wb2_d[:, 1])
            nc.sync.dma_start(out=wb2[:, 2], in_=wb2_d[:, 2])
        else:
            # plan E: per-t ws chunks alternating SP/Pool so the m1_A chain
            # streams at DMA pitch; slot0 split across both queues after.
            for t in range(0, 9, 2):
                nc.sync.dma_start(out=ws[:, t], in_=ws_d[:, t])
            for t in range(1, 9, 2):
                nc.gpsimd.dma_start(out=ws[:, t], in_=ws_d[:, t])
            nc.scalar.dma_start(out=ui, in_=ui_d[:, :])
            nc.scalar.dma_start(out=id64, in_=id64_d[:, :])
            nc.sync.dma_start(out=wb2[:, 0, 0:36], in_=wb2_d[:, 0, 0:36])
            nc.gpsimd.dma_start(out=wb2[:, 0, 36:72], in_=wb2_d[:, 0, 36:72])
            nc.scalar.dma_start(out=ui2, in_=ui2_d[:, :])
            nc.sync.dma_start(out=id128, in_=id128_d[:, :])
            nc.sync.dma_start(out=wb2[:, 1], in_=wb2_d[:, 1])
            nc.gpsimd.dma_start(out=wb2[:, 2], in_=wb2_d[:, 2])

        # persistent work tiles
        e = work.tile([128, O, T, B], BF16, name="e")
        rz_f = work.tile([128, T, B], F32, name="rz_f")
        rzb = work.tile([128, T, B], BF16, name="rzb")
        v_f = work.tile([64, O, J], F32, name="v_f")
        v_b = work.tile([64, O, J], BF16, name="v_b")
        v_jb = work.tile([128, 3, B], BF16, name="v_jb")
        s2 = work.tile([64, O, J], F32, name="s2")
        sq = work.tile([64, O], F32, name="sq")
        t1 = work.tile([64, O], F32, name="t1")
        den = work.tile([64, O], F32, name="den")
        rcp = work.tile([64, O], F32, name="rcp")
        ff = work.tile([64, O], F32, name="ff")

        s_ps = psS.tile([64, O, J], F32, name="s_ps")

        def m1_A_chain(h):
            """it0 half h: c uniform -> s_raw[b, o-half] = sum_{i,d} W u."""
            o5 = slice(5 * h, 5 * h + 5)
            for t in range(T):
                for d in range(D):
                    td = t * D + d
                    nc.tensor.matmul(
                        s_ps[:, o5, :].rearrange("p o j -> p (o j)"),
                        ui[:, t, d, :],
                        ws[:, t, d, o5, :].rearrange("p o j -> p (o j)"),
                        start=(td == 0), stop=(td == KC - 1),
                        tile_position=(0, 0), skip_group_check=True,
                    )

        def squash(it, h):
            """v[:, half] = squash(scale * s_ps[:, half]), tiny b-part ops."""
            scale = 0.1 if it == 0 else 1.0
            o5 = slice(5 * h, 5 * h + 5)
            nc.scalar.activation(s2[:, o5, :], s_ps[:, o5, :], ACTF.Square,
                                 scale=scale)
            nc.vector.tensor_reduce(sq[:, o5], s2[:, o5, :], axis=AX.X,
                                    op=ALU.add)
            nc.scalar.activation(t1[:, o5], sq[:, o5], ACTF.Ln, bias=eps1)
            nc.scalar.activation(den[:, o5], t1[:, o5], ACTF.Exp, scale=0.5)
            nc.vector.tensor_scalar_add(t1[:, o5], sq[:, o5], 1.0)
            nc.vector.tensor_tensor(den[:, o5], den[:, o5], t1[:, o5],
                                    op=ALU.mult)
            nc.vector.reciprocal(rcp[:, o5], den[:, o5])
            nc.vector.tensor_tensor(ff[:, o5], sq[:, o5], rcp[:, o5],
                                    op=ALU.mult)
            if it == 0:
                nc.vector.tensor_scalar_mul(ff[:, o5], ff[:, o5], scale)
            nc.vector.tensor_tensor(
                v_f[:, o5, :], s_ps[:, o5, :],
                ff[:, o5].unsqueeze(2).broadcast_to([64, 5, J]),
                op=ALU.mult)
            nc.vector.tensor_copy(
                v_b[:, o5, :].rearrange("p o j -> p (o j)"),
                v_f[:, o5, :].rearrange("p o j -> p (o j)"))

        # half-h (g, sl) slots are disjoint: h0 -> sl0 strips + (g0, sl1);
        # h1 -> (g1..3, sl1) + (g0..1, sl2).
        def transposes(h):
            vt = psVT.tile([128, 3, B], BF16, name="vt", tag="vt0",
                           bufs=1)
            for o in range(5 * h, 5 * h + 5):
                g, sl = o % 4, o // 4
                nc.tensor.matmul(
                    vt[32 * g : 32 * g + 16, sl, :],
                    v_b[:, o, :], id64,
                    is_transpose=True, tile_position=(0, 32 * g),
                )
            for o in range(5 * h, 5 * h + 5):
                g, sl = o % 4, o // 4
                if VJB_ENG == "a":
                    nc.scalar.copy(v_jb[32 * g : 32 * g + 16, sl, :],
                                   vt[32 * g : 32 * g + 16, sl, :])
                else:
                    nc.vector.tensor_copy(
                        v_jb[32 * g : 32 * g + 16, sl, :],
                        vt[32 * g : 32 * g + 16, sl, :])

        flat = lambda ap: ap.rearrange("p t b -> p (t b)")
        flat3 = lambda ap: ap.rearrange("p a b -> p (a b)")

        def emit_G(o, route):
            """G^T chunks for o; returns the ug tile being filled."""
            g, sl = o % 4, o // 4
            ug = ugp.tile([128, KC, B], BF16, name="ug", tag="ug")
            gsb = None
            if route != "b":
                gsb = gsbp.tile([128, KC, B], BF16, name="gsb", tag="gsb")

            def gmm(pg, kk, kc):
                nc.tensor.matmul(
                    pg[:, kk, :],
                    wb2[32 * g : 32 * g + 16, sl, kc, :],
                    v_jb[32 * g : 32 * g + 16, sl, :],
                    start=True, stop=True,
                    tile_position=(32 * g, 0),
                )

            if route == "6":
                # 16-kc psG tiles: (Act, Act, DVE, DVE, Act-half); Pool
                # multiplies the Act-copied parts.
                for ti in range(5):
                    k0 = 16 * ti
                    nk = 16 if ti < 4 else 8
                    pg = psG.tile([128, 16, B], F32, name="pg", tag="pg")
                    for kk in range(nk):
                        gmm(pg, kk, k0 + kk)
                    slk = slice(k0, k0 + nk)
                    if ti in (0, 1, 4):
                        nc.scalar.copy(flat3(gsb[:, slk, :]),
                                       flat3(pg[:, 0:nk, :]))
                    else:
                        nc.vector.tensor_tensor(
                            flat3(ug[:, slk, :]), flat3(pg[:, 0:nk, :]),
                            flat3(ui2[:, slk, :]), op=ALU.mult)
                nc.gpsimd.tensor_tensor(
                    flat3(ug[:, 0:32, :]), flat3(gsb[:, 0:32, :]),
                    flat3(ui2[:, 0:32, :]), op=ALU.mult)
                nc.gpsimd.tensor_tensor(
                    flat3(ug[:, 64:72, :]), flat3(gsb[:, 64:72, :]),
                    flat3(ui2[:, 64:72, :]), op=ALU.mult)
                return ug

            nA = int(M_H[o]) if route in ("m", "M") else 9
            for h in range(9):
                pg = psG.tile([128, 8, B], F32, name="pg", tag="pg")
                for kk in range(8):
                    gmm(pg, kk, 8 * h + kk)
                sl8 = slice(8 * h, 8 * h + 8)
                if route in ("a", "A") or (route in ("m", "M") and h < nA):
                    nc.scalar.copy(flat3(gsb[:, sl8, :]),
                                   flat3(pg[:, 0:8, :]))
                else:  # DVE straight from PSUM
                    nc.vector.tensor_tensor(
                        flat3(ug[:, sl8, :]), flat3(pg[:, 0:8, :]),
                        flat3(ui2[:, sl8, :]), op=ALU.mult)
            if route in ("a", "A"):
                meng = nc.gpsimd if route == "A" else nc.vector
                meng.tensor_tensor(flat3(ug), flat3(gsb), flat3(ui2),
                                   op=ALU.mult)
            elif route in ("m", "M"):
                meng = nc.gpsimd if route == "M" else nc.vector
                if SPLIT_MULT and nA >= 3:
                    # two halves so the first can run while the later Act
                    # copies are still in flight
                    cut = 8 * (nA // 2 + 1)
                    for sl_ in (slice(0, cut), slice(cut, 8 * nA)):
                        meng.tensor_tensor(
                            flat3(ug[:, sl_, :]), flat3(gsb[:, sl_, :]),
                            flat3(ui2[:, sl_, :]), op=ALU.mult)
                else:
                    hA = slice(0, 8 * nA)
                    meng.tensor_tensor(
                        flat3(ug[:, hA, :]), flat3(gsb[:, hA, :]),
                        flat3(ui2[:, hA, :]), op=ALU.mult)
            return ug

        def emit_ds(o, ug):
            """delta[o] = sum_d ug chunks.  Returns (psum_tile|None, ug)."""
            eng = DS_ENG[o]
            if eng == "T":  # PE identity-matmul chains into PSUM f32
                ds = psDS.tile([128, T, B], F32, name="ds", tag="ds")
                for d in range(D):
                    nc.tensor.matmul(
                        flat3(ds[:, 0:8, :]), id128,
                        flat3(ug[:, d * T : d * T + 8, :]),
                        start=(d == 0), stop=(d == D - 1),
                        tile_position=(0, 0), skip_group_check=True,
                    )
                for d in range(D):
                    nc.tensor.matmul(
                        ds[:, 8, :], id128, ug[:, d * T + 8, :],
                        start=(d == 0), stop=(d == D - 1),
                        tile_position=(0, 0), skip_group_check=True,
                    )
                return ds, ug
            ve = nc.vector if eng == "v" else nc.gpsimd
            # in-place bf16 fold tree: 72 -> 36 -> 18 -> 9 chunks
            for w in (36, 18, 9):
                ve.tensor_tensor(flat3(ug[:, 0:w, :]), flat3(ug[:, 0:w, :]),
                                 flat3(ug[:, w : 2 * w, :]), op=ALU.add)
            return None, ug

        def emit_exp(o, dsug, r):
            ds, ug = dsug
            src = flat3(ds) if ds is not None else flat3(ug[:, 0:T, :])
            if r == 0:
                nc.scalar.activation(flat(e[:, o]), src, ACTF.Exp)
            else:
                et = etp.tile([128, T, B], BF16, name="et", tag="et")
                nc.scalar.activation(flat(et), src, ACTF.Exp)
                eeng = nc.gpsimd if E_ENG[o] == "g" else nc.vector
                eeng.tensor_tensor(flat(e[:, o]), flat(e[:, o]),
                                   flat(et), op=ALU.mult)

        def emit_zpair(q):
            """partial softmax sums on Pool, overlapped with m2."""
            zq = zp.tile([128, T, B], BF16, name="zq", tag=f"z{q}", bufs=1)
            nc.gpsimd.tensor_tensor(flat(zq), flat(e[:, 2 * q]),
                                    flat(e[:, 2 * q + 1]), op=ALU.add)
            _zpart.append(zq)
            if q in (1, 3):  # fold pairs into quads as soon as available
                zz = zp.tile([128, T, B], BF16, name="zz", tag=f"zz{q}",
                             bufs=1)
                nc.gpsimd.tensor_tensor(flat(zz), flat(_zpart[-2]),
                                        flat(_zpart[-1]), op=ALU.add)
                _zquad.append(zz)

        DS_SHIFT = int(os.environ.get("DS_SHIFT", "3"))

        def m2(r, it):
            """delta_o for all o -> e (pass r), software-pipelined.
            Caller has emitted squash(it,0)+transposes(0); squash/transposes
            of the second half are interleaved after G(1)."""
            ugs = {}
            dss = {}
            for step in range(O + DS_SHIFT + 1):
                if step < O:
                    ugs[step] = emit_G(step, ROUTES[step])
                if step == 1:
                    squash(it, 1)
                    transposes(1)
                if 0 <= step - DS_SHIFT < O:
                    dss[step - DS_SHIFT] = emit_ds(
                        step - DS_SHIFT, ugs.pop(step - DS_SHIFT))
                if 0 <= step - DS_SHIFT - 1 < O:
                    oo = step - DS_SHIFT - 1
                    emit_exp(oo, dss.pop(oo), r)
                    if oo % 2 == 1:
                        emit_zpair(oo // 2)
                if step == 3 and len(_zpart) >= 2:
                    pass

        def softmax_tail():
            """finish Z = sum_o e; rz = 1/Z (bf16)."""
            za = zp.tile([128, T, B], BF16, name="za", tag="za")
            nc.vector.tensor_tensor(flat(za), flat(_zquad[0]),
                                    flat(_zquad[1]), op=ALU.add)
            nc.vector.tensor_tensor(flat(za), flat(za), flat(_zpart[4]),
                                    op=ALU.add)
            with nc.allow_low_precision("softmax normalizer, 2e-2 tolerance"):
                nc.vector.reciprocal(flat(rzb), flat(za))
            _zpart.clear()
            _zquad.clear()

        def m1_B(it):
            """s[b, o, j] = sum_{i,d} (c_o u) W for all o.
            squash/transposes of half 0 are emitted after o=4's chain."""
            for o in range(O):
                co = cop.tile([128, T, B], BF16, name="co", tag="co")
                ceng = nc.gpsimd if C_ENG[(it - 1) * O + o] == "g" else nc.vector
                ceng.tensor_tensor(flat(co), flat(e[:, o]), flat(rzb),
                                   op=ALU.mult)
                cu = cup.tile([128, T, D, B], BF16, name="cu", tag="cu")
                cueng = nc.gpsimd if CU_ENG[(it - 1) * O + o] == "g" else nc.vector
                cueng.tensor_tensor(
                    cu[:, :, :, :],
                    co.unsqueeze(2).broadcast_to([128, T, D, B]),
                    ui[:, :, :, :], op=ALU.mult)
                for t in range(T):
                    for d in range(D):
                        td = t * D + d
                        nc.tensor.matmul(
                            s_ps[:, o, :], cu[:, t, d, :], ws[:, t, d, o, :],
                            start=(td == 0), stop=(td == KC - 1),
                            tile_position=(0, 0), skip_group_check=True,
                        )
                if o == 4:
                    squash(it, 0)
                    if it < 2:
                        transposes(0)
                    else:
                        nc.sync.dma_start(out=vout_d[:, 0:5, :],
                                          in_=v_f[:, 0:5, :])

        _zpart = []
        _zquad = []

        # ========================= flow =========================
        W1 = int(os.environ.get("WARM1", "0"))
        W2 = int(os.environ.get("WARM2", "0"))
        m1_A_chain(0)
        squash(0, 0)
        m1_A_chain(1)
        transposes(0)
        for r in range(2):
            m2(r, r)
            pe_keepalive(W1)
            softmax_tail()
            m1_B(r + 1)
            pe_keepalive(W2)
        squash(2, 1)
        nc.sync.dma_start(out=vout_d[:, 5:10, :], in_=v_f[:, 5:10, :])

    nc.finalize()
    return nc


def _host_prep(u, weights):
    """Per-core input maps. u [512,1152,8] f32, weights [1152,10,16,8] f32."""
    W = np.asarray(weights, dtype=np.float32)
    u = np.asarray(u, dtype=np.float32)
    # ws[p, t, d, o, j] = W[t*128+p, o, j, d]
    ws = np.ascontiguousarray(
        W.reshape(T, 128, O, J, D).transpose(1, 0, 4, 2, 3)
    ).astype(bfnp)
    # wb2[32g+jj, sl, kc, m] = W[c*128+m, o, jj, d], kc = d*T + c
    wt = W.reshape(T, 128, O, J, D)  # [c, m, o, j, d]
    wb2 = np.zeros((128, 3, KC, 128), dtype=bfnp)
    for o in range(O):
        g, sl = o % 4, o // 4
        blk = wt[:, :, o, :, :].transpose(2, 3, 0, 1)  # [j, d, c, m]
        wb2[32 * g : 32 * g + 16, sl] = blk.reshape(J, KC, 128).astype(bfnp)
    id128 = np.eye(128, dtype=np.float32).astype(bfnp)
    id64 = np.eye(64, dtype=np.float32).astype(bfnp)

    base = {"ws": ws, "wb2": wb2, "id128": id128, "id64": id64}
    in_maps = []
    for c in range(N_CORES):
        uc = u[c * B : (c + 1) * B]  # [64, 1152, 8]
        ur = uc.reshape(B, T, 128, D)
        ui = np.ascontiguousarray(ur.transpose(2, 1, 3, 0)).astype(bfnp)
        # ui2[p, kc, b] = u[b, c*128+p, d], kc = d*T + c
        ui2 = np.ascontiguousarray(
            ur.transpose(2, 3, 1, 0).reshape(128, D * T, B)
        ).astype(bfnp)
        in_maps.append({**base, "ui": ui, "ui2": ui2})
    return in_maps


def kernel(u, weights):
    if "nc" not in _cache:
        _cache["nc"] = build_nc()
    nc = _cache["nc"]
    in_maps = _host_prep(u, weights)
    res = run_bass_kernel_spmd(nc, in_maps, core_ids=list(range(N_CORES)))
    out = np.concatenate([res.results[c]["vout"] for c in range(N_CORES)], axis=0)
    return out.astype(np.float32)


if __name__ == "__main__":
    rng = np.random.default_rng(0)
    u = rng.standard_normal((512, 1152, 8), dtype=np.float32)
    w = (rng.standard_normal((1152, 10, 16, 8)) * 0.1).astype(np.float32)
    v = kernel(u, w)
    print("out", v.shape, v.dtype, np.abs(v).max())
